# revision 14
# baseline (speedup 1.0000x reference)
"""CMDenNet Trainium2 kernel: host prep (FPS/kNN/z-order/gathers) + 3 SPMD Bass launches.

L1: enhancer MLPs, 8-way row split (896 points x 32 neighbors per core).
L2: encoders (4 bidirectional-Mamba blocks + pooling + out MLP); one unit per
    core, zero-padded to a uniform L=2052 so a single static program serves all
    cores (6 real units + 2 dummy cores).
L3: heads (g1/g2/g3 MLPs replicated, dec projections column-split 8 ways).

All data-dependent indexing (FPS, kNN, Morton order) is computed on host and
folded into the input packing; device kernels are shape-static, so programs
compile once per process and are reused across calls.
"""
import sys
import numpy as np

sys.path.insert(0, '/opt/trn_rl_repo')

import ml_dtypes
import concourse.bass as bass
import concourse.tile as tile
from concourse import bacc, mybir
from concourse import bass2jax

F32 = mybir.dt.float32
BF16 = mybir.dt.bfloat16
AF = mybir.ActivationFunctionType
ALU = mybir.AluOpType
AX = mybir.AxisListType
BF = ml_dtypes.bfloat16

H = 128; G = 512; K_NN = 32; RES = 64; S = 16; R = 8; KCONV = 4
LOW, MID, HIGH = 512, 1024, 2048
B, T, P, PD = 2, 16, 256, 6
NCORES = 8
LSEQ = 2052
NPTS = (LOW + MID + HIGH) * B          # 7168
PTS_PER_CORE = NPTS // NCORES          # 896
PAIRS_PER_CORE = PTS_PER_CORE * K_NN   # 28672
SGRP = 2                               # s-states per SSM group
CHS = [(0, 512), (512, 512), (1024, 512), (1536, 512), (2048, 4)]
CPC_L, CPC_M, CPC_H = LOW * 3 // NCORES, MID * 3 // NCORES, HIGH * 3 // NCORES


# ---------------------------------------------------------------- host math
def _fps(blocks, n):
    b, t, pp, d = blocks.shape
    pts = blocks.reshape(b * t, pp, d)
    xyz = pts[..., :3]
    mind = np.sum((xyz - xyz[:, :1]) ** 2, -1)
    idxs = [np.zeros((b * t,), np.int32)]
    for _ in range(n - 1):
        nxt = np.argmax(mind, axis=1)
        sel = np.take_along_axis(xyz, nxt[:, None, None], axis=1)
        mind = np.minimum(mind, np.sum((xyz - sel) ** 2, -1))
        idxs.append(nxt.astype(np.int32))
    idx = np.stack(idxs, 0).T
    out = np.take_along_axis(pts, idx[..., None], axis=1)
    return out.reshape(b, t * n, d)


def _knn(xyz):
    bsz, n, _ = xyz.shape
    sq = np.sum(xyz * xyz, -1)
    out = np.empty((bsz, n, K_NN), np.int64)
    for bi in range(bsz):
        d2 = sq[bi][:, None] + sq[bi][None, :] - 2.0 * (xyz[bi] @ xyz[bi].T)
        idx = np.argsort(d2, axis=1, kind='stable')[:, :K_NN + 1]
        out[bi] = idx[:, 1:]
    return out


def _zorder(xyz):
    mn = xyz.min(axis=1, keepdims=True)
    mx = xyz.max(axis=1, keepdims=True)
    g = np.clip(((xyz - mn) / (mx - mn + 1e-9) * RES).astype(np.int32), 0, RES - 1)
    code = np.zeros(g.shape[:2], np.int32)
    for bit in range(6):
        for ax in range(3):
            code = code | (((g[..., ax] >> bit) & 1) << (3 * bit + ax))
    return np.argsort(code, axis=1, kind='stable')


def _expand(pts, out_n):
    rep = -(-out_n // pts.shape[1])
    return np.repeat(pts, rep, axis=1)[:, :out_n]


def _np_tree(tree):
    if isinstance(tree, dict):
        return {k: _np_tree(v) for k, v in tree.items()}
    if isinstance(tree, (list, tuple)):
        return [_np_tree(v) for v in tree]
    return np.asarray(tree)


def _bf(x):
    return np.ascontiguousarray(np.asarray(x, np.float32).astype(BF))


def _f32(x):
    return np.ascontiguousarray(np.asarray(x, np.float32))


def _col(x):
    return _f32(np.asarray(x).reshape(128, 1))


# ---------------------------------------------------------------- bass helpers
def _mk_nc():
    return bacc.Bacc("TRN2", target_bir_lowering=False, debug=False,
                     num_devices=NCORES)


def rev_ap(ap_in):
    (pstep, pcnt), (estep, ecnt) = ap_in.ap
    return bass.AP(tensor=ap_in.tensor, offset=ap_in.offset + (ecnt - 1) * estep,
                   ap=[[pstep, pcnt], [-estep, ecnt]])


def rep_ap(ap_in, n):
    """[P, L] viewed as [P, n, L] with the middle axis stride 0."""
    (pstep, pcnt), (estep, ecnt) = ap_in.ap
    return bass.AP(tensor=ap_in.tensor, offset=ap_in.offset,
                   ap=[[pstep, pcnt], [0, n], [estep, ecnt]])


_RUNNERS = {}


def _runner(key, build_fn):
    if key in _RUNNERS:
        return _RUNNERS[key]
    import jax
    from jax.sharding import Mesh, PartitionSpec
    from jax.experimental.shard_map import shard_map

    nc = build_fn()
    bass2jax.install_neuronx_cc_hook()
    partition_name = nc.partition_id_tensor.name if nc.partition_id_tensor else None
    in_names, out_names, out_avals, zero_shapes = [], [], [], []
    for alloc in nc.m.functions[0].allocations:
        if not isinstance(alloc, mybir.MemoryLocationSet):
            continue
        name = alloc.memorylocations[0].name
        if alloc.kind == "ExternalInput":
            if name != partition_name:
                in_names.append(name)
        elif alloc.kind == "ExternalOutput":
            out_names.append(name)
            shape = tuple(alloc.tensor_shape)
            dtype = mybir.dt.np(alloc.dtype)
            out_avals.append(jax.core.ShapedArray(shape, dtype))
            zero_shapes.append((shape, dtype))
    n_params = len(in_names)
    n_outs = len(out_avals)
    in_names_all = list(in_names) + list(out_names)
    if partition_name is not None:
        in_names_all.append(partition_name)

    def _body(*args):
        operands = list(args)
        if partition_name is not None:
            operands.append(bass2jax.partition_id_tensor())
        outs = bass2jax._bass_exec_p.bind(
            *operands, out_avals=tuple(out_avals), in_names=tuple(in_names_all),
            out_names=tuple(out_names), lowering_input_output_aliases=(),
            sim_require_finite=False, sim_require_nnan=False, nc=nc)
        return tuple(outs)

    donate = tuple(range(n_params, n_params + n_outs))
    devices = jax.devices()[:NCORES]
    mesh = Mesh(np.asarray(devices), ("core",))
    jfn = jax.jit(shard_map(_body, mesh=mesh,
                            in_specs=(PartitionSpec("core"),) * (n_params + n_outs),
                            out_specs=(PartitionSpec("core"),) * n_outs,
                            check_rep=False),
                  donate_argnums=donate, keep_unused=True)

    def run(in_maps):
        per_core = [[np.ascontiguousarray(m[n]) for n in in_names] for m in in_maps]
        concat_in = [np.concatenate([per_core[c][i] for c in range(NCORES)], axis=0)
                     for i in range(n_params)]
        concat_zero = [np.zeros((NCORES * s[0], *s[1:]), d) for s, d in zero_shapes]
        outs = jfn(*concat_in, *concat_zero)
        outs = [np.asarray(o) for o in outs]
        return [{name: outs[i].reshape(NCORES, *out_avals[i].shape)[c]
                 for i, name in enumerate(out_names)} for c in range(NCORES)]

    _RUNNERS[key] = run
    return run


# ================================================================ LAUNCH 1
def _build_l1():
    NPAIR, NPT, CH = PAIRS_PER_CORE, PTS_PER_CORE, 512
    nc = _mk_nc()
    pr_in = nc.declare_dram_parameter("pr", [36, NPAIR], BF16, isOutput=False)
    names = [("w1c", [3, 128]), ("w1n", [4, 128]), ("w2c", [128, 128]),
             ("w2n", [128, 128]), ("w3a", [128, 128]), ("w3b", [128, 128]),
             ("w4", [128, 128])]
    dins = {n: nc.declare_dram_parameter(n, s, BF16, isOutput=False) for n, s in names}
    enh_out = nc.declare_dram_parameter("enh", [128, NPT], F32, isOutput=True)

    with tile.TileContext(nc) as tc:
        with tc.tile_pool(name="w", bufs=1) as wp, \
             tc.tile_pool(name="mid", bufs=3) as mp, \
             tc.tile_pool(name="ps", bufs=3, space="PSUM") as pp:
            WT = {}
            for n, s in names:
                if n == "w1n":
                    w1n_t = wp.tile([36, 128], BF16, tag=n, name="t_" + n)
                    nc.sync.dma_start(out=w1n_t[32:36, :], in_=dins[n][:])
                    WT[n] = w1n_t
                else:
                    WT[n] = wp.tile(s, BF16, tag=n, name="t_" + n)
                    nc.sync.dma_start(out=WT[n][:], in_=dins[n][:])
            pr = wp.tile([36, NPAIR], BF16, tag="pr")
            nc.sync.dma_start(out=pr[:], in_=pr_in[:])
            cf = wp.tile([128, NPT], BF16, tag="cf")
            nf = wp.tile([128, NPT], BF16, tag="nf")

            for j0 in range(0, NPAIR, CH):
                npair = min(CH, NPAIR - j0)
                npt = npair // K_NN
                p0 = j0 // K_NN
                for bk in ("c", "n"):
                    rows = pr[0:3, j0:j0 + npair] if bk == "c" else pr[32:36, j0:j0 + npair]
                    w1 = WT["w1c"][:] if bk == "c" else WT["w1n"][32:36, :]
                    ps1 = pp.tile([128, CH], F32, tag="mm")
                    nc.tensor.matmul(ps1[:, :npair], w1, rows,
                                     start=True, stop=True)
                    a1 = mp.tile([128, CH], BF16, tag="a1")
                    nc.scalar.activation(a1[:, :npair], ps1[:, :npair], AF.Gelu)
                    ps2 = pp.tile([128, CH], F32, tag="mm")
                    nc.tensor.matmul(ps2[:, :npair], WT["w2" + bk][:], a1[:, :npair],
                                     start=True, stop=True)
                    dst = cf if bk == "c" else nf
                    nc.vector.tensor_reduce(
                        dst[:, p0:p0 + npt],
                        ps2[:, :npair].rearrange("p (n k) -> p n k", k=K_NN),
                        axis=AX.X, op=ALU.max)
            for j0 in range(0, NPT, CH):
                npt = min(CH, NPT - j0)
                ps3 = pp.tile([128, CH], F32, tag="mm")
                nc.tensor.matmul(ps3[:, :npt], WT["w3a"][:], cf[:, j0:j0 + npt],
                                 start=True, stop=False)
                nc.tensor.matmul(ps3[:, :npt], WT["w3b"][:], nf[:, j0:j0 + npt],
                                 start=False, stop=True)
                a3 = mp.tile([128, CH], BF16, tag="a3")
                nc.scalar.activation(a3[:, :npt], ps3[:, :npt], AF.Gelu)
                ps4 = pp.tile([128, CH], F32, tag="mm")
                nc.tensor.matmul(ps4[:, :npt], WT["w4"][:], a3[:, :npt],
                                 start=True, stop=True)
                o = mp.tile([128, CH], F32, tag="o")
                nc.scalar.copy(o[:, :npt], ps4[:, :npt])
                nc.sync.dma_start(out=enh_out[:, j0:j0 + npt], in_=o[:, :npt])
    nc.compile()
    return nc


# ================================================================ LAUNCH 2
_L2_WNAMES = []


def _l2_weight_decls():
    names = [("xa", [128, LSEQ], BF16), ("xb", [6, LSEQ], BF16),
             ("mask", [128, LSEQ], BF16), ("nmask", [128, LSEQ], BF16),
             ("smask", [128, LSEQ], BF16),
             ("invn", [128, 1], F32),
             ("win_a", [128, 128], BF16), ("win_b", [6, 128], BF16)]
    for k in range(4):
        names += [(f"ln_g{k}", [128, 1], F32), (f"ln_b{k}", [128, 1], F32),
                  (f"wim{k}", [128, 256], BF16), (f"cw{k}", [128, 4], F32),
                  (f"cb{k}", [128, 1], F32), (f"xpw{k}", [128, 40], BF16),
                  (f"dtw{k}", [8, 128], BF16), (f"dtbn{k}", [128, 1], F32),
                  (f"dp2{k}", [128, 1], F32), (f"wout{k}", [128, 128], BF16),
                  (f"wob{k}", [128, 1], F32)]
    names += [("norm_g", [128, 1], F32), ("norm_b", [128, 1], F32),
              ("sel", [40, 32 * 128], BF16),
              ("pw1a", [128, 512], BF16), ("pw1b", [128, 512], BF16),
              ("pw2", [128, 2048], BF16)]
    return names


def _layernorm(nc, ap_, sm, pp, seq, xh, g, b, ones, eps, onesf):
    L = LSEQ
    mu = sm.tile([1, L], F32, tag="ln_mu")
    mu2 = sm.tile([1, L], F32, tag="ln_mu2")
    std = sm.tile([1, L], F32, tag="ln_std")
    sq = ap_.tile([128, L], F32, tag="cacc")          # alias cacc slot
    nc.scalar.activation(sq[:], seq[:], AF.Square)
    for c0, cw in CHS:
        sl = slice(c0, c0 + cw)
        pm = pp.tile([1, 512], F32, tag="red")
        nc.tensor.matmul(pm[:, :cw], ones[:], seq[:, sl], start=True, stop=True)
        nc.vector.tensor_copy(mu[:, sl], pm[:, :cw])
        pm2 = pp.tile([1, 512], F32, tag="red")
        nc.tensor.matmul(pm2[:, :cw], ones[:], sq[:, sl], start=True, stop=True)
        nc.vector.tensor_copy(mu2[:, sl], pm2[:, :cw])
    nc.vector.tensor_mul(std[:], mu[:], mu[:])
    nc.vector.tensor_sub(mu2[:], mu2[:], std[:])
    nc.scalar.activation(std[:], mu2[:], AF.Sqrt, bias=eps[:])
    rstd = mu2
    nc.vector.reciprocal(rstd[:], std[:])
    for c0, cw in CHS:
        sl = slice(c0, c0 + cw)
        mub = pp.tile([128, 512], F32, tag="mm", name="mub")
        nc.tensor.matmul(mub[:, :cw], onesf[:], mu[:, sl], start=True, stop=True)
        nc.vector.tensor_sub(xh[:, sl], seq[:, sl], mub[:, :cw])
        rstdb = pp.tile([128, 512], F32, tag="mm", name="rstdb")
        nc.tensor.matmul(rstdb[:, :cw], onesf[:], rstd[:, sl], start=True, stop=True)
        nc.vector.tensor_mul(xh[:, sl], xh[:, sl], rstdb[:, :cw])
    nc.vector.tensor_scalar(out=xh[:], in0=xh[:], scalar1=g[:], scalar2=b[:],
                            op0=ALU.mult, op1=ALU.add)


def _mamba_block(nc, ap_, gp, sm, pp, WT, seq, ones, sel, k):
    L = LSEQ
    xh = ap_.tile([128, L], F32, tag="xh")
    _layernorm(nc, ap_, sm, pp, seq, xh, WT[f"ln_g{k}"], WT[f"ln_b{k}"], ones, WT["eps_"], WT["onesf_"])
    xh_bf = ap_.tile([128, L], BF16, tag="ybf")       # alias ybf slot
    nc.vector.tensor_copy(xh_bf[:], xh[:])
    xi = ap_.tile([128, L], BF16, tag="xi")
    sz = ap_.tile([128, L], BF16, tag="sz")
    for c0, cw in CHS:
        sl = slice(c0, c0 + cw)
        p1 = pp.tile([128, 512], F32, tag="mm")
        nc.tensor.matmul(p1[:, :cw], WT[f"wim{k}"][:, 0:128], xh_bf[:, sl],
                         start=True, stop=True)
        nc.scalar.copy(xi[:, sl], p1[:, :cw])
        p2 = pp.tile([128, 512], F32, tag="mm")
        nc.tensor.matmul(p2[:, :cw], WT[f"wim{k}"][:, 128:256], xh_bf[:, sl],
                         start=True, stop=True)
        nc.scalar.activation(sz[:, sl], p2[:, :cw], AF.Silu)
    cacc = ap_.tile([128, L], F32, tag="cacc")
    cw_t = WT[f"cw{k}"]
    nc.vector.tensor_scalar(out=cacc[:], in0=xi[:], scalar1=cw_t[:, 3:4],
                            scalar2=WT[f"cb{k}"][:], op0=ALU.mult, op1=ALU.add)
    for j in range(3):
        sh = 3 - j
        nc.vector.scalar_tensor_tensor(cacc[:, sh:], xi[:, :L - sh],
                                       cw_t[:, j:j + 1], cacc[:, sh:],
                                       ALU.mult, ALU.add)
    xic = ap_.tile([128, L], BF16, tag="xic")
    nc.scalar.activation(xic[:], cacc[:], AF.Silu)
    dbc_bf = ap_.tile([40, L], BF16, tag="dbc_bf")
    for c0, cw in CHS:
        sl = slice(c0, c0 + cw)
        p3 = pp.tile([40, 512], F32, tag="mm")
        nc.tensor.matmul(p3[:, :cw], WT[f"xpw{k}"][:], xic[:, sl],
                         start=True, stop=True)
        nc.scalar.copy(dbc_bf[:, sl], p3[:, :cw])
    # dt = softplus(p) via sigmoid/ln: dt_f holds ln(sigmoid(-p)) = -dt
    sig = ap_.tile([128, L], F32, tag="y", name="sig")
    dt_f = ap_.tile([128, L], F32, tag="dt_f")
    for c0, cw in CHS:
        sl = slice(c0, c0 + cw)
        p4 = pp.tile([128, 512], F32, tag="mm")
        nc.tensor.matmul(p4[:, :cw], WT[f"dtw{k}"][:], dbc_bf[0:8, sl],
                         start=True, stop=True)
        nc.scalar.activation(sig[:, sl], p4[:, :cw], AF.Sigmoid,
                             scale=-1.0, bias=WT[f"dtbn{k}"][:])
    nc.scalar.activation(dt_f[:], sig[:], AF.Ln)
    u = ap_.tile([128, L], BF16, tag="u")
    dt_bf = ap_.tile([128, L], BF16, tag="dt_bf")
    nc.vector.tensor_scalar_mul(dt_bf[:], dt_f[:], -1.0)
    nc.vector.tensor_mul(u[:], dt_bf[:], xic[:])
    nc.vector.tensor_mul(u[:], u[:], WT["smask"][:])
    y = ap_.tile([128, L], F32, tag="y")
    fl = lambda t: t.rearrange("p s l -> p (s l)")
    for g in range(S // SGRP):
        dA_f = gp.tile([128, SGRP, L], BF16, tag="dA_f")
        dA_b = gp.tile([128, SGRP, L], BF16, tag="dA_b")
        dBx = gp.tile([128, SGRP, L], BF16, tag="dBx")
        hf = gp.tile([128, SGRP, L], BF16, tag="hf")
        hb = gp.tile([128, SGRP, L], BF16, tag="dA_f", name="hb")
        Bb = gp.tile([128, SGRP, L], BF16, tag="Bb")
        Cb = gp.tile([128, SGRP, L], BF16, tag="Cb")
        for si in range(SGRP):
            s = g * SGRP + si
            nc.scalar.activation(dA_f[:, si, :], dt_f[:], AF.Exp,
                                 scale=float(s + 1))
            for c0, cw in CHS:
                sl = slice(c0, c0 + cw)
                pb = pp.tile([128, 512], F32, tag="mm")
                nc.tensor.matmul(pb[:, :cw], sel[:, s * 128:(s + 1) * 128],
                                 dbc_bf[0:40, sl], start=True, stop=True)
                nc.scalar.copy(Bb[:, si, sl], pb[:, :cw])
                pc = pp.tile([128, 512], F32, tag="mm")
                nc.tensor.matmul(pc[:, :cw], sel[:, (16 + s) * 128:(17 + s) * 128],
                                 dbc_bf[0:40, sl], start=True, stop=True)
                nc.scalar.copy(Cb[:, si, sl], pc[:, :cw])
        nc.scalar.copy(dA_b[:], dA_f[:])
        nc.vector.memset(dA_f[:, :, 0:1], 0.0)
        nc.vector.memset(dA_b[:, :, L - 1:L], 0.0)
        nc.vector.tensor_mul(dBx[:], rep_ap(u[:], SGRP), Bb[:])
        nc.vector.tensor_tensor_scan(fl(hf[:]), fl(dA_f[:]), fl(dBx[:]),
                                     0.0, ALU.mult, ALU.add)
        nc.vector.tensor_tensor_scan(rev_ap(fl(hb[:])), rev_ap(fl(dA_b[:])),
                                     rev_ap(fl(dBx[:])), 0.0, ALU.mult, ALU.add)
        nc.gpsimd.tensor_add(hf[:], hf[:], hb[:])
        nc.vector.tensor_mul(hf[:], hf[:], Cb[:])
        if g == 0:
            nc.vector.tensor_add(y[:], hf[:, 0, :], hf[:, 1, :])
        else:
            nc.vector.tensor_add(hf[:, 0, :], hf[:, 0, :], hf[:, 1, :])
            nc.vector.tensor_add(y[:], y[:], hf[:, 0, :])
    nc.vector.scalar_tensor_tensor(y[:], xic[:], WT[f"dp2{k}"][:], y[:],
                                   ALU.mult, ALU.add)
    ybf = ap_.tile([128, L], BF16, tag="ybf")
    nc.vector.tensor_mul(ybf[:], y[:], sz[:])
    for c0, cw in CHS:
        sl = slice(c0, c0 + cw)
        p5 = pp.tile([128, 512], F32, tag="mm")
        nc.tensor.matmul(p5[:, :cw], WT[f"wout{k}"][:], ybf[:, sl],
                         start=True, stop=True)
        nc.vector.scalar_tensor_tensor(seq[:, sl], p5[:, :cw], WT[f"wob{k}"][:],
                                       seq[:, sl], ALU.add, ALU.add)


def _build_l2():
    L = LSEQ
    nc = _mk_nc()
    decls = _l2_weight_decls()
    dp = {n: nc.declare_dram_parameter(n, s, d, isOutput=False)
          for n, s, d in decls}
    eo_out = nc.declare_dram_parameter("eo", [128, 4], F32, isOutput=True)

    with tile.TileContext(nc) as tc:
        with tc.tile_pool(name="w", bufs=1) as wp, \
             tc.tile_pool(name="act", bufs=1) as ap_, \
             tc.tile_pool(name="grp", bufs=1) as gp, \
             tc.tile_pool(name="sm", bufs=1) as sm, \
             tc.tile_pool(name="ps", bufs=3, space="PSUM") as pp:
            WT = {}
            for n, s, d in decls:
                WT[n] = wp.tile(list(s), d, tag=n, name="t_" + n)
                nc.sync.dma_start(out=WT[n][:], in_=dp[n][:])
            ones = wp.tile([128, 1], F32, tag="ones_")
            nc.vector.memset(ones[:], 1.0 / 128.0)
            eps_t = wp.tile([1, 1], F32, tag="eps_")
            nc.vector.memset(eps_t[:], 1e-5)
            WT["eps_"] = eps_t
            onesf = wp.tile([1, 128], F32, tag="onesf_")
            nc.vector.memset(onesf[:], 1.0)
            WT["onesf_"] = onesf
            sel = WT["sel"]

            seq = wp.tile([128, L], F32, tag="seq")
            for c0, cw in CHS:
                sl = slice(c0, c0 + cw)
                pst = pp.tile([128, 512], F32, tag="mm")
                nc.tensor.matmul(pst[:, :cw], WT["win_a"][:], WT["xa"][:, sl],
                                 start=True, stop=False)
                nc.tensor.matmul(pst[:, :cw], WT["win_b"][:], WT["xb"][:, sl],
                                 start=False, stop=True)
                nc.scalar.copy(seq[:, sl], pst[:, :cw])

            for k in range(4):
                _mamba_block(nc, ap_, gp, sm, pp, WT, seq, ones, sel, k)

            xh = ap_.tile([128, L], F32, tag="xh")
            _layernorm(nc, ap_, sm, pp, seq, xh, WT["norm_g"], WT["norm_b"], ones, WT["eps_"], WT["onesf_"])
            xm = ap_.tile([128, L], F32, tag="y")
            nc.vector.scalar_tensor_tensor(xm[:], xh[:], 1.0, WT["nmask"][:],
                                           ALU.mult, ALU.add)
            pmax = sm.tile([128, 1], F32, tag="pmax")
            nc.vector.tensor_reduce(pmax[:], xm[:], axis=AX.X, op=ALU.max)
            xs = ap_.tile([128, L], F32, tag="cacc")
            nc.vector.tensor_mul(xs[:], xh[:], WT["mask"][:])
            psm = sm.tile([128, 1], F32, tag="psm")
            nc.vector.tensor_reduce(psm[:], xs[:], axis=AX.X, op=ALU.add)
            pmean = sm.tile([128, 1], F32, tag="pmean")
            nc.vector.tensor_mul(pmean[:], psm[:], WT["invn"][:])
            pool_bf = sm.tile([128, 2], BF16, tag="pool_bf")
            nc.vector.tensor_copy(pool_bf[:, 0:1], pmax[:])
            nc.vector.tensor_copy(pool_bf[:, 1:2], pmean[:])
            ps_m = pp.tile([128, 4], F32, tag="mm")
            for m in range(4):
                nc.tensor.matmul(ps_m[:, m:m + 1],
                                 WT["pw1a"][:, m * 128:(m + 1) * 128],
                                 pool_bf[:, 0:1], start=True, stop=False)
                nc.tensor.matmul(ps_m[:, m:m + 1],
                                 WT["pw1b"][:, m * 128:(m + 1) * 128],
                                 pool_bf[:, 1:2], start=False, stop=True)
            h1 = sm.tile([128, 4], BF16, tag="h1")
            nc.scalar.activation(h1[:], ps_m[:], AF.Gelu)
            ps_o = pp.tile([128, 4], F32, tag="mm")
            for m in range(4):
                for kc in range(4):
                    nc.tensor.matmul(
                        ps_o[:, m:m + 1],
                        WT["pw2"][:, kc * 512 + m * 128:kc * 512 + (m + 1) * 128],
                        h1[:, kc:kc + 1], start=(kc == 0), stop=(kc == 3))
            eo_t = sm.tile([128, 4], F32, tag="eo_t")
            nc.scalar.copy(eo_t[:], ps_o[:])
            nc.sync.dma_start(out=eo_out[:], in_=eo_t[:])
    nc.compile()
    return nc


# ================================================================ LAUNCH 3
def _build_l3():
    nc = _mk_nc()
    din = {}
    def D(name, shape, dt=BF16):
        din[name] = nc.declare_dram_parameter(name, shape, dt, isOutput=False)
    D("gfT", [128, 24])
    D("gw1", [128, 3 * 12 * 512])
    D("gw2", [128, 3 * 4 * 512])
    D("dwl", [128, 4 * CPC_L]); D("dwm", [128, 4 * CPC_M]); D("dwh", [128, 4 * CPC_H])
    outs = {"ol": nc.declare_dram_parameter("ol", [CPC_L, 2], F32, isOutput=True),
            "om": nc.declare_dram_parameter("om", [CPC_M, 2], F32, isOutput=True),
            "oh": nc.declare_dram_parameter("oh", [CPC_H, 2], F32, isOutput=True)}

    with tile.TileContext(nc) as tc:
        with tc.tile_pool(name="w", bufs=1) as wp, \
             tc.tile_pool(name="sm", bufs=1) as sm, \
             tc.tile_pool(name="ps", bufs=2, space="PSUM") as pp:
            WT = {}
            for name, t in din.items():
                WT[name] = wp.tile(list(t.shape), t.dtype, tag=name, name="t_" + name)
                nc.sync.dma_start(out=WT[name][:], in_=t[:])
            f2s = []
            for i in range(3):
                psf = pp.tile([128, 4, 2], F32, tag="mm")
                for m in range(4):
                    for kc in range(12):
                        w = WT["gw1"][:, ((i * 12 + kc) * 4 + m) * 128:
                                      ((i * 12 + kc) * 4 + m + 1) * 128]
                        nc.tensor.matmul(psf[:, m, :], w,
                                         WT["gfT"][:, kc * 2:(kc + 1) * 2],
                                         start=(kc == 0), stop=(kc == 11))
                f_bf = wp.tile([128, 4, 2], BF16, tag=f"f_bf{i}")
                nc.scalar.activation(f_bf[:], psf[:], AF.Gelu)
                ps2 = pp.tile([128, 4, 2], F32, tag="mm")
                for m in range(4):
                    for kc in range(4):
                        w = WT["gw2"][:, ((i * 4 + kc) * 4 + m) * 128:
                                      ((i * 4 + kc) * 4 + m + 1) * 128]
                        nc.tensor.matmul(ps2[:, m, :], w, f_bf[:, kc, :],
                                         start=(kc == 0), stop=(kc == 3))
                f2 = wp.tile([128, 4, 2], BF16, tag=f"f2_{i}")
                nc.vector.tensor_copy(f2[:], ps2[:])
                f2s.append(f2)
            # dec: low <- f3 (i=2), mid <- f2 (i=1), high <- f1 (i=0)
            for nm, cpc, fi in (("l", CPC_L, 2), ("m", CPC_M, 1), ("h", CPC_H, 0)):
                mcs = [(m0, min(128, cpc - m0)) for m0 in range(0, cpc, 128)]
                for m0, mw in mcs:
                    psd = pp.tile([128, 2], F32, tag="mm")
                    for kc in range(4):
                        w = WT["dw" + nm][:, kc * cpc + m0: kc * cpc + m0 + mw]
                        nc.tensor.matmul(psd[:mw, :], w, f2s[fi][:, kc, :],
                                         start=(kc == 0), stop=(kc == 3))
                    ot = sm.tile([128, 2], F32, tag="ot")
                    nc.scalar.copy(ot[:mw, :], psd[:mw, :])
                    nc.sync.dma_start(out=outs["o" + nm][m0:m0 + mw, :],
                                      in_=ot[:mw, :])
    nc.compile()
    return nc


# ================================================================ host packing
def _prep_unit(sampled_b, enhT_b, order_b, win):
    """Build xa/xb/mask/nmask/invn for one (b,res) unit."""
    n = sampled_b.shape[0]
    xin = np.zeros((134, LSEQ), np.float32)
    xin[0:6, 1:n + 1] = sampled_b[order_b].T
    xin[6:134, 1:n + 1] = enhT_b
    # oip_e column: v solving win.T @ v = oip_e
    return xin


def _pack_l2_core(xin, v_e, n, enc_p):
    L = LSEQ
    xin = xin.copy()
    xin[:, n + 1] = v_e
    mask = np.zeros((128, L), np.float32); mask[:, 1:n + 1] = 1.0
    smask = np.zeros((128, L), np.float32); smask[:, 0:n + 2] = 1.0
    nmask = np.full((128, L), -1e9, np.float32); nmask[:, 1:n + 1] = 0.0
    m = {"xa": _bf(xin[0:128]), "xb": _bf(xin[128:134]),
         "mask": _bf(mask), "nmask": _bf(nmask), "smask": _bf(smask),
         "invn": _f32(np.full((128, 1), 1.0 / n))}
    w_in = _f32(enc_p['in']['w'])
    m["win_a"] = _bf(w_in[0:128]); m["win_b"] = _bf(w_in[128:134])
    for k, blk in enumerate(enc_p['blocks']):
        m[f"ln_g{k}"] = _col(blk['ln_g']); m[f"ln_b{k}"] = _col(blk['ln_b'])
        m[f"wim{k}"] = _bf(blk['in']['w'])
        m[f"cw{k}"] = _f32(blk['conv_w']); m[f"cb{k}"] = _col(blk['conv_b'])
        m[f"xpw{k}"] = _bf(blk['xp_w'])
        m[f"dtw{k}"] = _bf(blk['dt']['w'])       # [R=8, di=128] is lhsT already
        m[f"dtbn{k}"] = _col(-np.asarray(blk['dt']['b'], np.float32))
        m[f"dp2{k}"] = _col(2.0 * np.asarray(blk['Dp'], np.float32))
        m[f"wout{k}"] = _bf(blk['out']['w']); m[f"wob{k}"] = _col(blk['out']['b'])
    m["norm_g"] = _col(enc_p['norm_g']); m["norm_b"] = _col(enc_p['norm_b'])
    seln = np.zeros((40, 32 * 128), np.float32)
    for idx in range(32):
        seln[8 + idx, idx * 128:(idx + 1) * 128] = 1.0
    m["sel"] = _bf(seln)
    pw1 = _f32(enc_p['out']['l1']['w'])
    m["pw1a"] = _bf(pw1[0:128]); m["pw1b"] = _bf(pw1[128:256])
    pw2 = _f32(enc_p['out']['l2']['w'])
    m["pw2"] = _bf(np.concatenate([pw2[kc * 128:(kc + 1) * 128, :]
                                   for kc in range(4)], axis=1))
    return m


def _dummy_l2_core():
    z = {n: np.zeros(s, BF if d == BF16 else np.float32)
         for n, s, d in _l2_weight_decls()}
    z["invn"][:] = 1.0
    return z


def kernel(arch_points, arch_blocks, params):
    params = _np_tree(params)
    arch_blocks = np.asarray(arch_blocks, np.float32)

    res_cfg = [("low", LOW), ("mid", MID), ("high", HIGH)]
    units = []           # list of dicts in order (res, b)
    pair_feats = []      # [7, n*32] per unit
    for rname, total in res_cfg:
        per = max(1, total // T)
        sampled = _fps(arch_blocks, per)            # [B, N, 6]
        xyz, nrm = sampled[..., :3], sampled[..., 3:6]
        knn = _knn(xyz)
        order = _zorder(xyz)
        for b in range(B):
            o = order[b]
            nb = knn[b][o]                           # [N, 32] (z-ordered rows)
            rel = xyz[b][nb] - xyz[b][o][:, None, :]
            nnb = nrm[b][nb]
            var = np.abs(1.0 - np.sum(nrm[b][o][:, None, :] * nnb, -1))
            pf = np.concatenate([rel.reshape(-1, 3),
                                 nnb.reshape(-1, 3),
                                 var.reshape(-1, 1)], axis=1).T  # [7, N*32]
            pair_feats.append(pf.astype(np.float32))
            units.append({"res": rname, "b": b, "n": sampled.shape[1],
                          "sampled": sampled[b], "order": o})

    # ---- launch 1
    allpairs = np.concatenate(pair_feats, axis=1)    # [7, NPTS*32]
    run1 = _runner("l1", _build_l1)
    enh_p = params['enh']
    w1 = {"w1c": _bf(enh_p['coord']['l1']['w']),
          "w1n": _bf(enh_p['normal']['l1']['w']),
          "w2c": _bf(enh_p['coord']['l2']['w']),
          "w2n": _bf(enh_p['normal']['l2']['w']),
          "w3a": _bf(enh_p['out']['l1']['w'][0:128]),
          "w3b": _bf(enh_p['out']['l1']['w'][128:256]),
          "w4": _bf(enh_p['out']['l2']['w'])}
    in_maps1 = []
    for c in range(NCORES):
        sl = slice(c * PAIRS_PER_CORE, (c + 1) * PAIRS_PER_CORE)
        pr = np.zeros((36, PAIRS_PER_CORE), np.float32)
        pr[0:3] = allpairs[0:3, sl]
        pr[32:36] = allpairs[3:7, sl]
        in_maps1.append({"pr": _bf(pr), **w1})
    res1 = run1(in_maps1)
    enhT = np.concatenate([res1[c]["enh"] for c in range(NCORES)], axis=1)  # [128, NPTS]

    # ---- launch 2
    run2 = _runner("l2", _build_l2)
    in_maps2 = []
    off = 0
    enc_key = {"low": "enc_low", "mid": "enc_mid", "high": "enc_high"}
    for u in units:
        n = u["n"]
        enc_p = params[enc_key[u["res"]]]
        xin = _prep_unit(u["sampled"], enhT[:, off:off + n], u["order"],
                         enc_p['in']['w'])
        off += n
        oip_e = (np.asarray(enc_p['oip']['w'], np.float32)[0]
                 + np.asarray(enc_p['oip']['b'], np.float32))
        w_in = _f32(enc_p['in']['w'])
        v_e = np.linalg.lstsq(w_in.T, oip_e - _f32(enc_p['in']['b']),
                              rcond=None)[0]
        in_maps2.append(_pack_l2_core(xin, v_e, n, enc_p))
    while len(in_maps2) < NCORES:
        in_maps2.append(_dummy_l2_core())
    res2 = run2(in_maps2)
    eo = [res2[c]["eo"].T.flatten() for c in range(6)]   # [512] each
    # units order: low_b0, low_b1, mid_b0, mid_b1, high_b0, high_b1
    gf = np.stack([np.concatenate([eo[0], eo[2], eo[4]]),
                   np.concatenate([eo[1], eo[3], eo[5]])], axis=0)  # [2,1536]

    # ---- launch 3
    run3 = _runner("l3", _build_l3)
    gfT = np.zeros((128, 24), np.float32)
    for kc in range(12):
        gfT[:, kc * 2:(kc + 1) * 2] = gf[:, kc * 128:(kc + 1) * 128].T
    gw1 = np.zeros((128, 3 * 12 * 512), np.float32)
    for i, gk in enumerate(("g1", "g2", "g3")):
        W1 = _f32(params[gk]['l1']['w'])
        for kc in range(12):
            for mch in range(4):
                gw1[:, ((i * 12 + kc) * 4 + mch) * 128:((i * 12 + kc) * 4 + mch + 1) * 128] = \
                    W1[kc * 128:(kc + 1) * 128, mch * 128:(mch + 1) * 128]
    gw2 = np.zeros((128, 3 * 4 * 512), np.float32)
    for i, gk in enumerate(("g1", "g2", "g3")):
        W2 = _f32(params[gk]['l2']['w'])
        for kc in range(4):
            for mch in range(4):
                gw2[:, ((i * 4 + kc) * 4 + mch) * 128:((i * 4 + kc) * 4 + mch + 1) * 128] = \
                    W2[kc * 128:(kc + 1) * 128, mch * 128:(mch + 1) * 128]
    in_maps3 = []
    dec = params['dec']
    for c in range(NCORES):
        m = {"gfT": _bf(gfT), "gw1": _bf(gw1), "gw2": _bf(gw2)}
        for nm, key, cpc in (("dwl", "low", CPC_L), ("dwm", "mid", CPC_M),
                             ("dwh", "high", CPC_H)):
            W = _f32(dec[key]['w'])                  # [512, LOWx3]
            sl = W[:, c * cpc:(c + 1) * cpc]
            m[nm] = _bf(np.concatenate([sl[kc * 128:(kc + 1) * 128, :]
                                        for kc in range(4)], axis=1))
        in_maps3.append(m)
    res3 = run3(in_maps3)
    dlo = np.concatenate([res3[c]["ol"].T for c in range(NCORES)], axis=1)
    dmd = np.concatenate([res3[c]["om"].T for c in range(NCORES)], axis=1)
    dhi = np.concatenate([res3[c]["oh"].T for c in range(NCORES)], axis=1)

    pl = dlo.reshape(B, LOW, 3)
    pm = _expand(pl, MID) + dmd.reshape(B, MID, 3)
    ph = _expand(pm, HIGH) + dhi.reshape(B, HIGH, 3)
    return (pl.astype(np.float32), pm.astype(np.float32),
            ph.astype(np.float32), gf.astype(np.float32))


# revision 18
# speedup vs baseline: 1.1925x; 1.1925x over previous
"""CMDenNet Trainium2 kernel: host prep (FPS/kNN/z-order/gathers) + 3 SPMD Bass launches.

L1: enhancer MLPs, 8-way row split (896 points x 32 neighbors per core).
L2: encoders (4 bidirectional-Mamba blocks + pooling + out MLP); one unit per
    core, zero-padded to a uniform L=2052 so a single static program serves all
    cores (6 real units + 2 dummy cores).
L3: heads (g1/g2/g3 MLPs replicated, dec projections column-split 8 ways).

All data-dependent indexing (FPS, kNN, Morton order) is computed on host and
folded into the input packing; device kernels are shape-static, so programs
compile once per process and are reused across calls.
"""
import sys
import numpy as np

sys.path.insert(0, '/opt/trn_rl_repo')

import ml_dtypes
import concourse.bass as bass
import concourse.tile as tile
from concourse import bacc, mybir
from concourse import bass2jax

F32 = mybir.dt.float32
BF16 = mybir.dt.bfloat16
AF = mybir.ActivationFunctionType
ALU = mybir.AluOpType
AX = mybir.AxisListType
BF = ml_dtypes.bfloat16

H = 128; G = 512; K_NN = 32; RES = 64; S = 16; R = 8; KCONV = 4
LOW, MID, HIGH = 512, 1024, 2048
B, T, P, PD = 2, 16, 256, 6
NCORES = 8
LSEQ = 2052
NPTS = (LOW + MID + HIGH) * B          # 7168
PTS_PER_CORE = NPTS // NCORES          # 896
PAIRS_PER_CORE = PTS_PER_CORE * K_NN   # 28672
SGRP = 2                               # s-states per SSM group
CHS = [(0, 512), (512, 512), (1024, 512), (1536, 512), (2048, 4)]
CPC_L, CPC_M, CPC_H = LOW * 3 // NCORES, MID * 3 // NCORES, HIGH * 3 // NCORES


# ---------------------------------------------------------------- host math
def _fps(blocks, n):
    b, t, pp, d = blocks.shape
    pts = blocks.reshape(b * t, pp, d)
    xyz = pts[..., :3]
    mind = np.sum((xyz - xyz[:, :1]) ** 2, -1)
    idxs = [np.zeros((b * t,), np.int32)]
    for _ in range(n - 1):
        nxt = np.argmax(mind, axis=1)
        sel = np.take_along_axis(xyz, nxt[:, None, None], axis=1)
        mind = np.minimum(mind, np.sum((xyz - sel) ** 2, -1))
        idxs.append(nxt.astype(np.int32))
    idx = np.stack(idxs, 0).T
    out = np.take_along_axis(pts, idx[..., None], axis=1)
    return out.reshape(b, t * n, d)


def _knn(xyz):
    bsz, n, _ = xyz.shape
    sq = np.sum(xyz * xyz, -1)
    out = np.empty((bsz, n, K_NN), np.int64)
    for bi in range(bsz):
        d2 = sq[bi][:, None] + sq[bi][None, :] - 2.0 * (xyz[bi] @ xyz[bi].T)
        idx = np.argsort(d2, axis=1, kind='stable')[:, :K_NN + 1]
        out[bi] = idx[:, 1:]
    return out


def _zorder(xyz):
    mn = xyz.min(axis=1, keepdims=True)
    mx = xyz.max(axis=1, keepdims=True)
    g = np.clip(((xyz - mn) / (mx - mn + 1e-9) * RES).astype(np.int32), 0, RES - 1)
    code = np.zeros(g.shape[:2], np.int32)
    for bit in range(6):
        for ax in range(3):
            code = code | (((g[..., ax] >> bit) & 1) << (3 * bit + ax))
    return np.argsort(code, axis=1, kind='stable')


def _expand(pts, out_n):
    rep = -(-out_n // pts.shape[1])
    return np.repeat(pts, rep, axis=1)[:, :out_n]


def _np_tree(tree):
    if isinstance(tree, dict):
        return {k: _np_tree(v) for k, v in tree.items()}
    if isinstance(tree, (list, tuple)):
        return [_np_tree(v) for v in tree]
    return np.asarray(tree)


def _bf(x):
    return np.ascontiguousarray(np.asarray(x, np.float32).astype(BF))


def _f32(x):
    return np.ascontiguousarray(np.asarray(x, np.float32))


def _col(x):
    return _f32(np.asarray(x).reshape(128, 1))


# ---------------------------------------------------------------- bass helpers
def _mk_nc():
    return bacc.Bacc("TRN2", target_bir_lowering=False, debug=False,
                     num_devices=NCORES)


def rev_ap(ap_in):
    (pstep, pcnt), (estep, ecnt) = ap_in.ap
    return bass.AP(tensor=ap_in.tensor, offset=ap_in.offset + (ecnt - 1) * estep,
                   ap=[[pstep, pcnt], [-estep, ecnt]])


def rep_ap(ap_in, n):
    """[P, L] viewed as [P, n, L] with the middle axis stride 0."""
    (pstep, pcnt), (estep, ecnt) = ap_in.ap
    return bass.AP(tensor=ap_in.tensor, offset=ap_in.offset,
                   ap=[[pstep, pcnt], [0, n], [estep, ecnt]])


_RUNNERS = {}


def _runner(key, build_fn):
    if key in _RUNNERS:
        return _RUNNERS[key]
    import jax
    from jax.sharding import Mesh, PartitionSpec
    from jax.experimental.shard_map import shard_map

    nc = build_fn()
    bass2jax.install_neuronx_cc_hook()
    partition_name = nc.partition_id_tensor.name if nc.partition_id_tensor else None
    in_names, out_names, out_avals, zero_shapes = [], [], [], []
    for alloc in nc.m.functions[0].allocations:
        if not isinstance(alloc, mybir.MemoryLocationSet):
            continue
        name = alloc.memorylocations[0].name
        if alloc.kind == "ExternalInput":
            if name != partition_name:
                in_names.append(name)
        elif alloc.kind == "ExternalOutput":
            out_names.append(name)
            shape = tuple(alloc.tensor_shape)
            dtype = mybir.dt.np(alloc.dtype)
            out_avals.append(jax.core.ShapedArray(shape, dtype))
            zero_shapes.append((shape, dtype))
    n_params = len(in_names)
    n_outs = len(out_avals)
    in_names_all = list(in_names) + list(out_names)
    if partition_name is not None:
        in_names_all.append(partition_name)

    def _body(*args):
        operands = list(args)
        if partition_name is not None:
            operands.append(bass2jax.partition_id_tensor())
        outs = bass2jax._bass_exec_p.bind(
            *operands, out_avals=tuple(out_avals), in_names=tuple(in_names_all),
            out_names=tuple(out_names), lowering_input_output_aliases=(),
            sim_require_finite=False, sim_require_nnan=False, nc=nc)
        return tuple(outs)

    donate = tuple(range(n_params, n_params + n_outs))
    devices = jax.devices()[:NCORES]
    mesh = Mesh(np.asarray(devices), ("core",))
    jfn = jax.jit(shard_map(_body, mesh=mesh,
                            in_specs=(PartitionSpec("core"),) * (n_params + n_outs),
                            out_specs=(PartitionSpec("core"),) * n_outs,
                            check_rep=False),
                  donate_argnums=donate, keep_unused=True)

    def run(in_maps):
        per_core = [[np.ascontiguousarray(m[n]) for n in in_names] for m in in_maps]
        concat_in = [np.concatenate([per_core[c][i] for c in range(NCORES)], axis=0)
                     for i in range(n_params)]
        concat_zero = [np.zeros((NCORES * s[0], *s[1:]), d) for s, d in zero_shapes]
        outs = jfn(*concat_in, *concat_zero)
        outs = [np.asarray(o) for o in outs]
        return [{name: outs[i].reshape(NCORES, *out_avals[i].shape)[c]
                 for i, name in enumerate(out_names)} for c in range(NCORES)]

    _RUNNERS[key] = run
    return run


# ================================================================ LAUNCH 1
def _build_l1():
    NPAIR, NPT, CH = PAIRS_PER_CORE, PTS_PER_CORE, 512
    nc = _mk_nc()
    pr_in = nc.declare_dram_parameter("pr", [36, NPAIR], BF16, isOutput=False)
    names = [("w1c", [3, 128]), ("w1n", [4, 128]), ("w2c", [128, 128]),
             ("w2n", [128, 128]), ("w3a", [128, 128]), ("w3b", [128, 128]),
             ("w4", [128, 128])]
    dins = {n: nc.declare_dram_parameter(n, s, BF16, isOutput=False) for n, s in names}
    enh_out = nc.declare_dram_parameter("enh", [128, NPT], F32, isOutput=True)

    with tile.TileContext(nc) as tc:
        with tc.tile_pool(name="w", bufs=1) as wp, \
             tc.tile_pool(name="mid", bufs=3) as mp, \
             tc.tile_pool(name="ps", bufs=3, space="PSUM") as pp:
            WT = {}
            for n, s in names:
                if n == "w1n":
                    w1n_t = wp.tile([36, 128], BF16, tag=n, name="t_" + n)
                    nc.sync.dma_start(out=w1n_t[32:36, :], in_=dins[n][:])
                    WT[n] = w1n_t
                else:
                    WT[n] = wp.tile(s, BF16, tag=n, name="t_" + n)
                    nc.sync.dma_start(out=WT[n][:], in_=dins[n][:])
            pr = wp.tile([36, NPAIR], BF16, tag="pr")
            nc.sync.dma_start(out=pr[:], in_=pr_in[:])
            cf = wp.tile([128, NPT], BF16, tag="cf")
            nf = wp.tile([128, NPT], BF16, tag="nf")

            for j0 in range(0, NPAIR, CH):
                npair = min(CH, NPAIR - j0)
                npt = npair // K_NN
                p0 = j0 // K_NN
                for bk in ("c", "n"):
                    rows = pr[0:3, j0:j0 + npair] if bk == "c" else pr[32:36, j0:j0 + npair]
                    w1 = WT["w1c"][:] if bk == "c" else WT["w1n"][32:36, :]
                    ps1 = pp.tile([128, CH], F32, tag="mm")
                    nc.tensor.matmul(ps1[:, :npair], w1, rows,
                                     start=True, stop=True)
                    a1 = mp.tile([128, CH], BF16, tag="a1")
                    nc.scalar.activation(a1[:, :npair], ps1[:, :npair], AF.Gelu)
                    ps2 = pp.tile([128, CH], F32, tag="mm")
                    nc.tensor.matmul(ps2[:, :npair], WT["w2" + bk][:], a1[:, :npair],
                                     start=True, stop=True)
                    dst = cf if bk == "c" else nf
                    nc.vector.tensor_reduce(
                        dst[:, p0:p0 + npt],
                        ps2[:, :npair].rearrange("p (n k) -> p n k", k=K_NN),
                        axis=AX.X, op=ALU.max)
            for j0 in range(0, NPT, CH):
                npt = min(CH, NPT - j0)
                ps3 = pp.tile([128, CH], F32, tag="mm")
                nc.tensor.matmul(ps3[:, :npt], WT["w3a"][:], cf[:, j0:j0 + npt],
                                 start=True, stop=False)
                nc.tensor.matmul(ps3[:, :npt], WT["w3b"][:], nf[:, j0:j0 + npt],
                                 start=False, stop=True)
                a3 = mp.tile([128, CH], BF16, tag="a3")
                nc.scalar.activation(a3[:, :npt], ps3[:, :npt], AF.Gelu)
                ps4 = pp.tile([128, CH], F32, tag="mm")
                nc.tensor.matmul(ps4[:, :npt], WT["w4"][:], a3[:, :npt],
                                 start=True, stop=True)
                o = mp.tile([128, CH], F32, tag="o")
                nc.scalar.copy(o[:, :npt], ps4[:, :npt])
                nc.sync.dma_start(out=enh_out[:, j0:j0 + npt], in_=o[:, :npt])
    nc.compile()
    return nc


# ================================================================ LAUNCH 2
_L2_WNAMES = []


def _l2_weight_decls():
    names = [("xa", [128, LSEQ], BF16), ("xb", [6, LSEQ], BF16),
             ("mask", [128, LSEQ], BF16), ("nmask", [128, LSEQ], BF16),
             ("smask", [128, LSEQ], BF16),
             ("invn", [128, 1], F32),
             ("win_a", [128, 128], BF16), ("win_b", [6, 128], BF16)]
    for k in range(4):
        names += [(f"ln_g{k}", [128, 1], F32), (f"ln_b{k}", [128, 1], F32),
                  (f"wim{k}", [128, 256], BF16), (f"cw{k}", [128, 4], F32),
                  (f"cb{k}", [128, 1], F32), (f"xpw{k}", [128, 40], BF16),
                  (f"dtw{k}", [8, 128], BF16), (f"dtbn{k}", [128, 1], F32),
                  (f"dp2{k}", [128, 1], F32), (f"wout{k}", [128, 128], BF16),
                  (f"wob{k}", [128, 1], F32)]
    names += [("norm_g", [128, 1], F32), ("norm_b", [128, 1], F32),
              ("sel", [40, 32 * 128], BF16),
              ("pw1a", [128, 512], BF16), ("pw1b", [128, 512], BF16),
              ("pw2", [128, 2048], BF16)]
    return names


def _layernorm(nc, ap_, sm, pp, seq, xh, g, b, ones, eps, onesf):
    L = LSEQ
    mu = sm.tile([1, L], F32, tag="ln_mu")
    mu2 = sm.tile([1, L], F32, tag="ln_mu2")
    std = sm.tile([1, L], F32, tag="ln_std")
    sq = ap_.tile([128, L], F32, tag="cacc")          # alias cacc slot
    nc.scalar.activation(sq[:], seq[:], AF.Square)
    for c0, cw in CHS:
        sl = slice(c0, c0 + cw)
        pm = pp.tile([1, 512], F32, tag="red")
        nc.tensor.matmul(pm[:, :cw], ones[:], seq[:, sl], start=True, stop=True)
        nc.vector.tensor_copy(mu[:, sl], pm[:, :cw])
        pm2 = pp.tile([1, 512], F32, tag="red")
        nc.tensor.matmul(pm2[:, :cw], ones[:], sq[:, sl], start=True, stop=True)
        nc.vector.tensor_copy(mu2[:, sl], pm2[:, :cw])
    nc.vector.tensor_mul(std[:], mu[:], mu[:])
    nc.vector.tensor_sub(mu2[:], mu2[:], std[:])
    nc.scalar.activation(std[:], mu2[:], AF.Sqrt, bias=eps[:])
    rstd = mu2
    nc.vector.reciprocal(rstd[:], std[:])
    for c0, cw in CHS:
        sl = slice(c0, c0 + cw)
        mub = pp.tile([128, 512], F32, tag="mm", name="mub")
        nc.tensor.matmul(mub[:, :cw], onesf[:], mu[:, sl], start=True, stop=True)
        nc.vector.tensor_sub(xh[:, sl], seq[:, sl], mub[:, :cw])
        rstdb = pp.tile([128, 512], F32, tag="mm", name="rstdb")
        nc.tensor.matmul(rstdb[:, :cw], onesf[:], rstd[:, sl], start=True, stop=True)
        nc.vector.tensor_mul(xh[:, sl], xh[:, sl], rstdb[:, :cw])
    nc.vector.tensor_scalar(out=xh[:], in0=xh[:], scalar1=g[:], scalar2=b[:],
                            op0=ALU.mult, op1=ALU.add)


def _mamba_block(nc, ap_, gp, sm, pp, WT, seq, ones, sel, k):
    L = LSEQ
    xh = ap_.tile([128, L], F32, tag="xh")
    _layernorm(nc, ap_, sm, pp, seq, xh, WT[f"ln_g{k}"], WT[f"ln_b{k}"], ones, WT["eps_"], WT["onesf_"])
    xh_bf = ap_.tile([128, L], BF16, tag="ybf")       # alias ybf slot
    nc.vector.tensor_copy(xh_bf[:], xh[:])
    xi = ap_.tile([128, L], BF16, tag="xi")
    sz = ap_.tile([128, L], BF16, tag="sz")
    for c0, cw in CHS:
        sl = slice(c0, c0 + cw)
        p1 = pp.tile([128, 512], F32, tag="mm")
        nc.tensor.matmul(p1[:, :cw], WT[f"wim{k}"][:, 0:128], xh_bf[:, sl],
                         start=True, stop=True)
        nc.scalar.copy(xi[:, sl], p1[:, :cw])
        p2 = pp.tile([128, 512], F32, tag="mm")
        nc.tensor.matmul(p2[:, :cw], WT[f"wim{k}"][:, 128:256], xh_bf[:, sl],
                         start=True, stop=True)
        nc.scalar.activation(sz[:, sl], p2[:, :cw], AF.Silu)
    cacc = ap_.tile([128, L], F32, tag="cacc")
    cw_t = WT[f"cw{k}"]
    nc.vector.tensor_scalar(out=cacc[:], in0=xi[:], scalar1=cw_t[:, 3:4],
                            scalar2=WT[f"cb{k}"][:], op0=ALU.mult, op1=ALU.add)
    for j in range(3):
        sh = 3 - j
        nc.vector.scalar_tensor_tensor(cacc[:, sh:], xi[:, :L - sh],
                                       cw_t[:, j:j + 1], cacc[:, sh:],
                                       ALU.mult, ALU.add)
    xic = ap_.tile([128, L], BF16, tag="xic")
    nc.scalar.activation(xic[:], cacc[:], AF.Silu)
    dbc_bf = ap_.tile([40, L], BF16, tag="dbc_bf")
    for c0, cw in CHS:
        sl = slice(c0, c0 + cw)
        p3 = pp.tile([40, 512], F32, tag="mm")
        nc.tensor.matmul(p3[:, :cw], WT[f"xpw{k}"][:], xic[:, sl],
                         start=True, stop=True)
        nc.scalar.copy(dbc_bf[:, sl], p3[:, :cw])
    # dt = softplus(p) via sigmoid/ln: dt_f holds ln(sigmoid(-p)) = -dt
    sig = ap_.tile([128, L], F32, tag="y", name="sig")
    dt_f = ap_.tile([128, L], F32, tag="dt_f")
    for c0, cw in CHS:
        sl = slice(c0, c0 + cw)
        p4 = pp.tile([128, 512], F32, tag="mm")
        nc.tensor.matmul(p4[:, :cw], WT[f"dtw{k}"][:], dbc_bf[0:8, sl],
                         start=True, stop=True)
        nc.scalar.activation(sig[:, sl], p4[:, :cw], AF.Sigmoid,
                             scale=-1.0, bias=WT[f"dtbn{k}"][:])
    nc.scalar.activation(dt_f[:], sig[:], AF.Ln)
    u = ap_.tile([128, L], BF16, tag="u")
    nc.vector.tensor_scalar_mul(u[:], dt_f[:], -1.0)
    nc.vector.tensor_mul(u[:], u[:], xic[:])
    nc.vector.tensor_mul(u[:], u[:], WT["smask"][:])
    y = ap_.tile([128, L], F32, tag="y")
    fl = lambda t: t.rearrange("p s l -> p (s l)")
    for g in range(S // SGRP):
        dA_f = gp.tile([128, SGRP, L], BF16, tag="dA_f")
        dA_b = gp.tile([128, SGRP, L], BF16, tag="dA_b")
        dBx = gp.tile([128, SGRP, L], BF16, tag="dBx", bufs=2)
        hf = gp.tile([128, SGRP, L], BF16, tag="hf")
        hb = gp.tile([128, SGRP, L], BF16, tag="dA_f", name="hb")
        Bb = gp.tile([128, SGRP, L], BF16, tag="Bb", bufs=2)
        Cb = gp.tile([128, SGRP, L], BF16, tag="Cb", bufs=2)
        for si in range(SGRP):
            s = g * SGRP + si
            nc.scalar.activation(dA_f[:, si, :], dt_f[:], AF.Exp,
                                 scale=float(s + 1))
            for c0, cw in CHS:
                sl = slice(c0, c0 + cw)
                pb = pp.tile([128, 512], F32, tag="mm")
                nc.tensor.matmul(pb[:, :cw], sel[:, s * 128:(s + 1) * 128],
                                 dbc_bf[0:40, sl], start=True, stop=True)
                nc.scalar.copy(Bb[:, si, sl], pb[:, :cw])
                pc = pp.tile([128, 512], F32, tag="mm")
                nc.tensor.matmul(pc[:, :cw], sel[:, (16 + s) * 128:(17 + s) * 128],
                                 dbc_bf[0:40, sl], start=True, stop=True)
                nc.scalar.copy(Cb[:, si, sl], pc[:, :cw])
        nc.scalar.copy(dA_b[:], dA_f[:])
        nc.vector.memset(dA_f[:, :, 0:1], 0.0)
        nc.vector.memset(dA_b[:, :, L - 1:L], 0.0)
        nc.vector.tensor_mul(dBx[:], rep_ap(u[:], SGRP), Bb[:])
        nc.vector.tensor_tensor_scan(fl(hf[:]), fl(dA_f[:]), fl(dBx[:]),
                                     0.0, ALU.mult, ALU.add)
        nc.vector.tensor_tensor_scan(rev_ap(fl(hb[:])), rev_ap(fl(dA_b[:])),
                                     rev_ap(fl(dBx[:])), 0.0, ALU.mult, ALU.add)
        nc.gpsimd.tensor_add(hf[:], hf[:], hb[:])
        nc.vector.tensor_mul(hf[:], hf[:], Cb[:])
        if g == 0:
            nc.vector.tensor_add(y[:], hf[:, 0, :], hf[:, 1, :])
        else:
            nc.vector.tensor_add(hf[:, 0, :], hf[:, 0, :], hf[:, 1, :])
            nc.vector.tensor_add(y[:], y[:], hf[:, 0, :])
    nc.vector.scalar_tensor_tensor(y[:], xic[:], WT[f"dp2{k}"][:], y[:],
                                   ALU.mult, ALU.add)
    ybf = ap_.tile([128, L], BF16, tag="ybf")
    nc.vector.tensor_mul(ybf[:], y[:], sz[:])
    for c0, cw in CHS:
        sl = slice(c0, c0 + cw)
        p5 = pp.tile([128, 512], F32, tag="mm")
        nc.tensor.matmul(p5[:, :cw], WT[f"wout{k}"][:], ybf[:, sl],
                         start=True, stop=True)
        nc.vector.scalar_tensor_tensor(seq[:, sl], p5[:, :cw], WT[f"wob{k}"][:],
                                       seq[:, sl], ALU.add, ALU.add)


def _build_l2():
    L = LSEQ
    nc = _mk_nc()
    decls = _l2_weight_decls()
    dp = {n: nc.declare_dram_parameter(n, s, d, isOutput=False)
          for n, s, d in decls}
    eo_out = nc.declare_dram_parameter("eo", [128, 4], F32, isOutput=True)

    with tile.TileContext(nc) as tc:
        with tc.tile_pool(name="w", bufs=1) as wp, \
             tc.tile_pool(name="act", bufs=1) as ap_, \
             tc.tile_pool(name="grp", bufs=1) as gp, \
             tc.tile_pool(name="sm", bufs=1) as sm, \
             tc.tile_pool(name="ps", bufs=3, space="PSUM") as pp:
            WT = {}
            for n, s, d in decls:
                tag = n
                WT[n] = wp.tile(list(s), d, tag=tag, name="t_" + n)
                nc.sync.dma_start(out=WT[n][:], in_=dp[n][:])
            ones = wp.tile([128, 1], F32, tag="ones_")
            nc.vector.memset(ones[:], 1.0 / 128.0)
            eps_t = wp.tile([1, 1], F32, tag="eps_")
            nc.vector.memset(eps_t[:], 1e-5)
            WT["eps_"] = eps_t
            onesf = wp.tile([1, 128], F32, tag="onesf_")
            nc.vector.memset(onesf[:], 1.0)
            WT["onesf_"] = onesf
            sel = WT["sel"]

            seq = wp.tile([128, L], F32, tag="seq")
            for c0, cw in CHS:
                sl = slice(c0, c0 + cw)
                pst = pp.tile([128, 512], F32, tag="mm")
                nc.tensor.matmul(pst[:, :cw], WT["win_a"][:], WT["xa"][:, sl],
                                 start=True, stop=False)
                nc.tensor.matmul(pst[:, :cw], WT["win_b"][:], WT["xb"][:, sl],
                                 start=False, stop=True)
                nc.scalar.copy(seq[:, sl], pst[:, :cw])

            for k in range(4):
                _mamba_block(nc, ap_, gp, sm, pp, WT, seq, ones, sel, k)

            xh = ap_.tile([128, L], F32, tag="xh")
            _layernorm(nc, ap_, sm, pp, seq, xh, WT["norm_g"], WT["norm_b"], ones, WT["eps_"], WT["onesf_"])
            xm = ap_.tile([128, L], F32, tag="y")
            nc.vector.scalar_tensor_tensor(xm[:], xh[:], 1.0, WT["nmask"][:],
                                           ALU.mult, ALU.add)
            pmax = sm.tile([128, 1], F32, tag="pmax")
            nc.vector.tensor_reduce(pmax[:], xm[:], axis=AX.X, op=ALU.max)
            xs = ap_.tile([128, L], F32, tag="cacc")
            nc.vector.tensor_mul(xs[:], xh[:], WT["mask"][:])
            psm = sm.tile([128, 1], F32, tag="psm")
            nc.vector.tensor_reduce(psm[:], xs[:], axis=AX.X, op=ALU.add)
            pmean = sm.tile([128, 1], F32, tag="pmean")
            nc.vector.tensor_mul(pmean[:], psm[:], WT["invn"][:])
            pool_bf = sm.tile([128, 2], BF16, tag="pool_bf")
            nc.vector.tensor_copy(pool_bf[:, 0:1], pmax[:])
            nc.vector.tensor_copy(pool_bf[:, 1:2], pmean[:])
            ps_m = pp.tile([128, 4], F32, tag="mm")
            for m in range(4):
                nc.tensor.matmul(ps_m[:, m:m + 1],
                                 WT["pw1a"][:, m * 128:(m + 1) * 128],
                                 pool_bf[:, 0:1], start=True, stop=False)
                nc.tensor.matmul(ps_m[:, m:m + 1],
                                 WT["pw1b"][:, m * 128:(m + 1) * 128],
                                 pool_bf[:, 1:2], start=False, stop=True)
            h1 = sm.tile([128, 4], BF16, tag="h1")
            nc.scalar.activation(h1[:], ps_m[:], AF.Gelu)
            ps_o = pp.tile([128, 4], F32, tag="mm")
            for m in range(4):
                for kc in range(4):
                    nc.tensor.matmul(
                        ps_o[:, m:m + 1],
                        WT["pw2"][:, kc * 512 + m * 128:kc * 512 + (m + 1) * 128],
                        h1[:, kc:kc + 1], start=(kc == 0), stop=(kc == 3))
            eo_t = sm.tile([128, 4], F32, tag="eo_t")
            nc.scalar.copy(eo_t[:], ps_o[:])
            nc.sync.dma_start(out=eo_out[:], in_=eo_t[:])
    nc.compile()
    return nc


# ================================================================ LAUNCH 3
def _build_l3():
    nc = _mk_nc()
    din = {}
    def D(name, shape, dt=BF16):
        din[name] = nc.declare_dram_parameter(name, shape, dt, isOutput=False)
    D("gfT", [128, 24])
    D("gw1", [128, 3 * 12 * 512])
    D("gw2", [128, 3 * 4 * 512])
    D("dwl", [128, 4 * CPC_L]); D("dwm", [128, 4 * CPC_M]); D("dwh", [128, 4 * CPC_H])
    outs = {"ol": nc.declare_dram_parameter("ol", [CPC_L, 2], F32, isOutput=True),
            "om": nc.declare_dram_parameter("om", [CPC_M, 2], F32, isOutput=True),
            "oh": nc.declare_dram_parameter("oh", [CPC_H, 2], F32, isOutput=True)}

    with tile.TileContext(nc) as tc:
        with tc.tile_pool(name="w", bufs=1) as wp, \
             tc.tile_pool(name="sm", bufs=1) as sm, \
             tc.tile_pool(name="ps", bufs=2, space="PSUM") as pp:
            WT = {}
            for name, t in din.items():
                WT[name] = wp.tile(list(t.shape), t.dtype, tag=name, name="t_" + name)
                nc.sync.dma_start(out=WT[name][:], in_=t[:])
            f2s = []
            for i in range(3):
                psf = pp.tile([128, 4, 2], F32, tag="mm")
                for m in range(4):
                    for kc in range(12):
                        w = WT["gw1"][:, ((i * 12 + kc) * 4 + m) * 128:
                                      ((i * 12 + kc) * 4 + m + 1) * 128]
                        nc.tensor.matmul(psf[:, m, :], w,
                                         WT["gfT"][:, kc * 2:(kc + 1) * 2],
                                         start=(kc == 0), stop=(kc == 11))
                f_bf = wp.tile([128, 4, 2], BF16, tag=f"f_bf{i}")
                nc.scalar.activation(f_bf[:], psf[:], AF.Gelu)
                ps2 = pp.tile([128, 4, 2], F32, tag="mm")
                for m in range(4):
                    for kc in range(4):
                        w = WT["gw2"][:, ((i * 4 + kc) * 4 + m) * 128:
                                      ((i * 4 + kc) * 4 + m + 1) * 128]
                        nc.tensor.matmul(ps2[:, m, :], w, f_bf[:, kc, :],
                                         start=(kc == 0), stop=(kc == 3))
                f2 = wp.tile([128, 4, 2], BF16, tag=f"f2_{i}")
                nc.vector.tensor_copy(f2[:], ps2[:])
                f2s.append(f2)
            # dec: low <- f3 (i=2), mid <- f2 (i=1), high <- f1 (i=0)
            for nm, cpc, fi in (("l", CPC_L, 2), ("m", CPC_M, 1), ("h", CPC_H, 0)):
                mcs = [(m0, min(128, cpc - m0)) for m0 in range(0, cpc, 128)]
                for m0, mw in mcs:
                    psd = pp.tile([128, 2], F32, tag="mm")
                    for kc in range(4):
                        w = WT["dw" + nm][:, kc * cpc + m0: kc * cpc + m0 + mw]
                        nc.tensor.matmul(psd[:mw, :], w, f2s[fi][:, kc, :],
                                         start=(kc == 0), stop=(kc == 3))
                    ot = sm.tile([128, 2], F32, tag="ot")
                    nc.scalar.copy(ot[:mw, :], psd[:mw, :])
                    nc.sync.dma_start(out=outs["o" + nm][m0:m0 + mw, :],
                                      in_=ot[:mw, :])
    nc.compile()
    return nc


# ================================================================ host packing
def _prep_unit(sampled_b, enhT_b, order_b, win):
    """Build xa/xb/mask/nmask/invn for one (b,res) unit."""
    n = sampled_b.shape[0]
    xin = np.zeros((134, LSEQ), np.float32)
    xin[0:6, 1:n + 1] = sampled_b[order_b].T
    xin[6:134, 1:n + 1] = enhT_b
    # oip_e column: v solving win.T @ v = oip_e
    return xin


def _pack_l2_core(xin, v_e, n, enc_p):
    L = LSEQ
    xin = xin.copy()
    xin[:, n + 1] = v_e
    mask = np.zeros((128, L), np.float32); mask[:, 1:n + 1] = 1.0
    smask = np.zeros((128, L), np.float32); smask[:, 0:n + 2] = 1.0
    nmask = np.full((128, L), -1e9, np.float32); nmask[:, 1:n + 1] = 0.0
    m = {"xa": _bf(xin[0:128]), "xb": _bf(xin[128:134]),
         "mask": _bf(mask), "nmask": _bf(nmask), "smask": _bf(smask),
         "invn": _f32(np.full((128, 1), 1.0 / n))}
    w_in = _f32(enc_p['in']['w'])
    m["win_a"] = _bf(w_in[0:128]); m["win_b"] = _bf(w_in[128:134])
    for k, blk in enumerate(enc_p['blocks']):
        m[f"ln_g{k}"] = _col(blk['ln_g']); m[f"ln_b{k}"] = _col(blk['ln_b'])
        m[f"wim{k}"] = _bf(blk['in']['w'])
        m[f"cw{k}"] = _f32(blk['conv_w']); m[f"cb{k}"] = _col(blk['conv_b'])
        m[f"xpw{k}"] = _bf(blk['xp_w'])
        m[f"dtw{k}"] = _bf(blk['dt']['w'])       # [R=8, di=128] is lhsT already
        m[f"dtbn{k}"] = _col(-np.asarray(blk['dt']['b'], np.float32))
        m[f"dp2{k}"] = _col(2.0 * np.asarray(blk['Dp'], np.float32))
        m[f"wout{k}"] = _bf(blk['out']['w']); m[f"wob{k}"] = _col(blk['out']['b'])
    m["norm_g"] = _col(enc_p['norm_g']); m["norm_b"] = _col(enc_p['norm_b'])
    seln = np.zeros((40, 32 * 128), np.float32)
    for idx in range(32):
        seln[8 + idx, idx * 128:(idx + 1) * 128] = 1.0
    m["sel"] = _bf(seln)
    pw1 = _f32(enc_p['out']['l1']['w'])
    m["pw1a"] = _bf(pw1[0:128]); m["pw1b"] = _bf(pw1[128:256])
    pw2 = _f32(enc_p['out']['l2']['w'])
    m["pw2"] = _bf(np.concatenate([pw2[kc * 128:(kc + 1) * 128, :]
                                   for kc in range(4)], axis=1))
    return m


def _dummy_l2_core():
    z = {n: np.zeros(s, BF if d == BF16 else np.float32)
         for n, s, d in _l2_weight_decls()}
    z["invn"][:] = 1.0
    return z


def kernel(arch_points, arch_blocks, params):
    params = _np_tree(params)
    arch_blocks = np.asarray(arch_blocks, np.float32)

    res_cfg = [("low", LOW), ("mid", MID), ("high", HIGH)]
    units = []           # list of dicts in order (res, b)
    pair_feats = []      # [7, n*32] per unit
    for rname, total in res_cfg:
        per = max(1, total // T)
        sampled = _fps(arch_blocks, per)            # [B, N, 6]
        xyz, nrm = sampled[..., :3], sampled[..., 3:6]
        knn = _knn(xyz)
        order = _zorder(xyz)
        for b in range(B):
            o = order[b]
            nb = knn[b][o]                           # [N, 32] (z-ordered rows)
            rel = xyz[b][nb] - xyz[b][o][:, None, :]
            nnb = nrm[b][nb]
            var = np.abs(1.0 - np.sum(nrm[b][o][:, None, :] * nnb, -1))
            pf = np.concatenate([rel.reshape(-1, 3),
                                 nnb.reshape(-1, 3),
                                 var.reshape(-1, 1)], axis=1).T  # [7, N*32]
            pair_feats.append(pf.astype(np.float32))
            units.append({"res": rname, "b": b, "n": sampled.shape[1],
                          "sampled": sampled[b], "order": o})

    # ---- launch 1
    allpairs = np.concatenate(pair_feats, axis=1)    # [7, NPTS*32]
    run1 = _runner("l1", _build_l1)
    enh_p = params['enh']
    w1 = {"w1c": _bf(enh_p['coord']['l1']['w']),
          "w1n": _bf(enh_p['normal']['l1']['w']),
          "w2c": _bf(enh_p['coord']['l2']['w']),
          "w2n": _bf(enh_p['normal']['l2']['w']),
          "w3a": _bf(enh_p['out']['l1']['w'][0:128]),
          "w3b": _bf(enh_p['out']['l1']['w'][128:256]),
          "w4": _bf(enh_p['out']['l2']['w'])}
    in_maps1 = []
    for c in range(NCORES):
        sl = slice(c * PAIRS_PER_CORE, (c + 1) * PAIRS_PER_CORE)
        pr = np.zeros((36, PAIRS_PER_CORE), np.float32)
        pr[0:3] = allpairs[0:3, sl]
        pr[32:36] = allpairs[3:7, sl]
        in_maps1.append({"pr": _bf(pr), **w1})
    res1 = run1(in_maps1)
    enhT = np.concatenate([res1[c]["enh"] for c in range(NCORES)], axis=1)  # [128, NPTS]

    # ---- launch 2
    run2 = _runner("l2", _build_l2)
    in_maps2 = []
    off = 0
    enc_key = {"low": "enc_low", "mid": "enc_mid", "high": "enc_high"}
    for u in units:
        n = u["n"]
        enc_p = params[enc_key[u["res"]]]
        xin = _prep_unit(u["sampled"], enhT[:, off:off + n], u["order"],
                         enc_p['in']['w'])
        off += n
        oip_e = (np.asarray(enc_p['oip']['w'], np.float32)[0]
                 + np.asarray(enc_p['oip']['b'], np.float32))
        w_in = _f32(enc_p['in']['w'])
        v_e = np.linalg.lstsq(w_in.T, oip_e - _f32(enc_p['in']['b']),
                              rcond=None)[0]
        in_maps2.append(_pack_l2_core(xin, v_e, n, enc_p))
    while len(in_maps2) < NCORES:
        in_maps2.append(_dummy_l2_core())
    res2 = run2(in_maps2)
    eo = [res2[c]["eo"].T.flatten() for c in range(6)]   # [512] each
    # units order: low_b0, low_b1, mid_b0, mid_b1, high_b0, high_b1
    gf = np.stack([np.concatenate([eo[0], eo[2], eo[4]]),
                   np.concatenate([eo[1], eo[3], eo[5]])], axis=0)  # [2,1536]

    # ---- launch 3
    run3 = _runner("l3", _build_l3)
    gfT = np.zeros((128, 24), np.float32)
    for kc in range(12):
        gfT[:, kc * 2:(kc + 1) * 2] = gf[:, kc * 128:(kc + 1) * 128].T
    gw1 = np.zeros((128, 3 * 12 * 512), np.float32)
    for i, gk in enumerate(("g1", "g2", "g3")):
        W1 = _f32(params[gk]['l1']['w'])
        for kc in range(12):
            for mch in range(4):
                gw1[:, ((i * 12 + kc) * 4 + mch) * 128:((i * 12 + kc) * 4 + mch + 1) * 128] = \
                    W1[kc * 128:(kc + 1) * 128, mch * 128:(mch + 1) * 128]
    gw2 = np.zeros((128, 3 * 4 * 512), np.float32)
    for i, gk in enumerate(("g1", "g2", "g3")):
        W2 = _f32(params[gk]['l2']['w'])
        for kc in range(4):
            for mch in range(4):
                gw2[:, ((i * 4 + kc) * 4 + mch) * 128:((i * 4 + kc) * 4 + mch + 1) * 128] = \
                    W2[kc * 128:(kc + 1) * 128, mch * 128:(mch + 1) * 128]
    in_maps3 = []
    dec = params['dec']
    for c in range(NCORES):
        m = {"gfT": _bf(gfT), "gw1": _bf(gw1), "gw2": _bf(gw2)}
        for nm, key, cpc in (("dwl", "low", CPC_L), ("dwm", "mid", CPC_M),
                             ("dwh", "high", CPC_H)):
            W = _f32(dec[key]['w'])                  # [512, LOWx3]
            sl = W[:, c * cpc:(c + 1) * cpc]
            m[nm] = _bf(np.concatenate([sl[kc * 128:(kc + 1) * 128, :]
                                        for kc in range(4)], axis=1))
        in_maps3.append(m)
    res3 = run3(in_maps3)
    dlo = np.concatenate([res3[c]["ol"].T for c in range(NCORES)], axis=1)
    dmd = np.concatenate([res3[c]["om"].T for c in range(NCORES)], axis=1)
    dhi = np.concatenate([res3[c]["oh"].T for c in range(NCORES)], axis=1)

    pl = dlo.reshape(B, LOW, 3)
    pm = _expand(pl, MID) + dmd.reshape(B, MID, 3)
    ph = _expand(pm, HIGH) + dhi.reshape(B, HIGH, 3)
    return (pl.astype(np.float32), pm.astype(np.float32),
            ph.astype(np.float32), gf.astype(np.float32))


# revision 22
# speedup vs baseline: 1.2822x; 1.0752x over previous
"""CMDenNet Trainium2 kernel: host prep (FPS/kNN/z-order/gathers) + 3 SPMD Bass launches.

L1: enhancer MLPs, 8-way row split (896 points x 32 neighbors per core).
L2: encoders (4 bidirectional-Mamba blocks + pooling + out MLP); one unit per
    core, zero-padded to a uniform L=2052 so a single static program serves all
    cores (6 real units + 2 dummy cores).
L3: heads (g1/g2/g3 MLPs replicated, dec projections column-split 8 ways).

All data-dependent indexing (FPS, kNN, Morton order) is computed on host and
folded into the input packing; device kernels are shape-static, so programs
compile once per process and are reused across calls.
"""
import sys
import numpy as np

sys.path.insert(0, '/opt/trn_rl_repo')

import ml_dtypes
import concourse.bass as bass
import concourse.tile as tile
from concourse import bacc, mybir
from concourse import bass2jax

F32 = mybir.dt.float32
BF16 = mybir.dt.bfloat16
AF = mybir.ActivationFunctionType
ALU = mybir.AluOpType
AX = mybir.AxisListType
BF = ml_dtypes.bfloat16

H = 128; G = 512; K_NN = 32; RES = 64; S = 16; R = 8; KCONV = 4
LOW, MID, HIGH = 512, 1024, 2048
B, T, P, PD = 2, 16, 256, 6
NCORES = 8
LSEQ = 2052
NPTS = (LOW + MID + HIGH) * B          # 7168
PTS_PER_CORE = NPTS // NCORES          # 896
PAIRS_PER_CORE = PTS_PER_CORE * K_NN   # 28672
SGRP = 2                               # s-states per SSM group
CHS = [(0, 512), (512, 512), (1024, 512), (1536, 512), (2048, 4)]
CPC_L, CPC_M, CPC_H = LOW * 3 // NCORES, MID * 3 // NCORES, HIGH * 3 // NCORES


# ---------------------------------------------------------------- host math
def _fps(blocks, n):
    b, t, pp, d = blocks.shape
    pts = blocks.reshape(b * t, pp, d)
    xyz = pts[..., :3]
    mind = np.sum((xyz - xyz[:, :1]) ** 2, -1)
    idxs = [np.zeros((b * t,), np.int32)]
    for _ in range(n - 1):
        nxt = np.argmax(mind, axis=1)
        sel = np.take_along_axis(xyz, nxt[:, None, None], axis=1)
        mind = np.minimum(mind, np.sum((xyz - sel) ** 2, -1))
        idxs.append(nxt.astype(np.int32))
    idx = np.stack(idxs, 0).T
    out = np.take_along_axis(pts, idx[..., None], axis=1)
    return out.reshape(b, t * n, d)


def _knn(xyz):
    bsz, n, _ = xyz.shape
    sq = np.sum(xyz * xyz, -1)
    out = np.empty((bsz, n, K_NN), np.int64)
    for bi in range(bsz):
        d2 = sq[bi][:, None] + sq[bi][None, :] - 2.0 * (xyz[bi] @ xyz[bi].T)
        idx = np.argsort(d2, axis=1, kind='stable')[:, :K_NN + 1]
        out[bi] = idx[:, 1:]
    return out


def _zorder(xyz):
    mn = xyz.min(axis=1, keepdims=True)
    mx = xyz.max(axis=1, keepdims=True)
    g = np.clip(((xyz - mn) / (mx - mn + 1e-9) * RES).astype(np.int32), 0, RES - 1)
    code = np.zeros(g.shape[:2], np.int32)
    for bit in range(6):
        for ax in range(3):
            code = code | (((g[..., ax] >> bit) & 1) << (3 * bit + ax))
    return np.argsort(code, axis=1, kind='stable')


def _expand(pts, out_n):
    rep = -(-out_n // pts.shape[1])
    return np.repeat(pts, rep, axis=1)[:, :out_n]


def _np_tree(tree):
    if isinstance(tree, dict):
        return {k: _np_tree(v) for k, v in tree.items()}
    if isinstance(tree, (list, tuple)):
        return [_np_tree(v) for v in tree]
    return np.asarray(tree)


def _bf(x):
    return np.ascontiguousarray(np.asarray(x, np.float32).astype(BF))


def _f32(x):
    return np.ascontiguousarray(np.asarray(x, np.float32))


def _col(x):
    return _f32(np.asarray(x).reshape(128, 1))


# ---------------------------------------------------------------- bass helpers
def _mk_nc():
    return bacc.Bacc("TRN2", target_bir_lowering=False, debug=False,
                     num_devices=NCORES)


def rev_ap(ap_in):
    (pstep, pcnt), (estep, ecnt) = ap_in.ap
    return bass.AP(tensor=ap_in.tensor, offset=ap_in.offset + (ecnt - 1) * estep,
                   ap=[[pstep, pcnt], [-estep, ecnt]])


def rep_ap(ap_in, n):
    """[P, L] viewed as [P, n, L] with the middle axis stride 0."""
    (pstep, pcnt), (estep, ecnt) = ap_in.ap
    return bass.AP(tensor=ap_in.tensor, offset=ap_in.offset,
                   ap=[[pstep, pcnt], [0, n], [estep, ecnt]])


_RUNNERS = {}


def _runner(key, build_fn):
    if key in _RUNNERS:
        return _RUNNERS[key]
    import jax
    from jax.sharding import Mesh, PartitionSpec
    from jax.experimental.shard_map import shard_map

    nc = build_fn()
    bass2jax.install_neuronx_cc_hook()
    partition_name = nc.partition_id_tensor.name if nc.partition_id_tensor else None
    in_names, out_names, out_avals, zero_shapes = [], [], [], []
    for alloc in nc.m.functions[0].allocations:
        if not isinstance(alloc, mybir.MemoryLocationSet):
            continue
        name = alloc.memorylocations[0].name
        if alloc.kind == "ExternalInput":
            if name != partition_name:
                in_names.append(name)
        elif alloc.kind == "ExternalOutput":
            out_names.append(name)
            shape = tuple(alloc.tensor_shape)
            dtype = mybir.dt.np(alloc.dtype)
            out_avals.append(jax.core.ShapedArray(shape, dtype))
            zero_shapes.append((shape, dtype))
    n_params = len(in_names)
    n_outs = len(out_avals)
    in_names_all = list(in_names) + list(out_names)
    if partition_name is not None:
        in_names_all.append(partition_name)

    def _body(*args):
        operands = list(args)
        if partition_name is not None:
            operands.append(bass2jax.partition_id_tensor())
        outs = bass2jax._bass_exec_p.bind(
            *operands, out_avals=tuple(out_avals), in_names=tuple(in_names_all),
            out_names=tuple(out_names), lowering_input_output_aliases=(),
            sim_require_finite=False, sim_require_nnan=False, nc=nc)
        return tuple(outs)

    donate = tuple(range(n_params, n_params + n_outs))
    devices = jax.devices()[:NCORES]
    mesh = Mesh(np.asarray(devices), ("core",))
    jfn = jax.jit(shard_map(_body, mesh=mesh,
                            in_specs=(PartitionSpec("core"),) * (n_params + n_outs),
                            out_specs=(PartitionSpec("core"),) * n_outs,
                            check_rep=False),
                  donate_argnums=donate, keep_unused=True)

    def run(in_maps):
        per_core = [[np.ascontiguousarray(m[n]) for n in in_names] for m in in_maps]
        concat_in = [np.concatenate([per_core[c][i] for c in range(NCORES)], axis=0)
                     for i in range(n_params)]
        concat_zero = [np.zeros((NCORES * s[0], *s[1:]), d) for s, d in zero_shapes]
        outs = jfn(*concat_in, *concat_zero)
        outs = [np.asarray(o) for o in outs]
        return [{name: outs[i].reshape(NCORES, *out_avals[i].shape)[c]
                 for i, name in enumerate(out_names)} for c in range(NCORES)]

    _RUNNERS[key] = run
    return run


# ================================================================ LAUNCH 1
def _build_l1():
    NPAIR, NPT, CH = PAIRS_PER_CORE, PTS_PER_CORE, 512
    nc = _mk_nc()
    pr_in = nc.declare_dram_parameter("pr", [36, NPAIR], BF16, isOutput=False)
    names = [("w1c", [3, 128]), ("w1n", [4, 128]), ("w2c", [128, 128]),
             ("w2n", [128, 128]), ("w3a", [128, 128]), ("w3b", [128, 128]),
             ("w4", [128, 128])]
    dins = {n: nc.declare_dram_parameter(n, s, BF16, isOutput=False) for n, s in names}
    enh_out = nc.declare_dram_parameter("enh", [128, NPT], F32, isOutput=True)

    with tile.TileContext(nc) as tc:
        with tc.tile_pool(name="w", bufs=1) as wp, \
             tc.tile_pool(name="mid", bufs=3) as mp, \
             tc.tile_pool(name="ps", bufs=3, space="PSUM") as pp:
            WT = {}
            for n, s in names:
                if n == "w1n":
                    w1n_t = wp.tile([36, 128], BF16, tag=n, name="t_" + n)
                    nc.sync.dma_start(out=w1n_t[32:36, :], in_=dins[n][:])
                    WT[n] = w1n_t
                else:
                    WT[n] = wp.tile(s, BF16, tag=n, name="t_" + n)
                    nc.sync.dma_start(out=WT[n][:], in_=dins[n][:])
            pr = wp.tile([36, NPAIR], BF16, tag="pr")
            nc.sync.dma_start(out=pr[:], in_=pr_in[:])
            cf = wp.tile([128, NPT], BF16, tag="cf")
            nf = wp.tile([128, NPT], BF16, tag="nf")

            for j0 in range(0, NPAIR, CH):
                npair = min(CH, NPAIR - j0)
                npt = npair // K_NN
                p0 = j0 // K_NN
                for bk in ("c", "n"):
                    rows = pr[0:3, j0:j0 + npair] if bk == "c" else pr[32:36, j0:j0 + npair]
                    w1 = WT["w1c"][:] if bk == "c" else WT["w1n"][32:36, :]
                    ps1 = pp.tile([128, CH], F32, tag="mm")
                    nc.tensor.matmul(ps1[:, :npair], w1, rows,
                                     start=True, stop=True)
                    a1 = mp.tile([128, CH], BF16, tag="a1")
                    nc.scalar.activation(a1[:, :npair], ps1[:, :npair], AF.Gelu)
                    ps2 = pp.tile([128, CH], F32, tag="mm")
                    nc.tensor.matmul(ps2[:, :npair], WT["w2" + bk][:], a1[:, :npair],
                                     start=True, stop=True)
                    dst = cf if bk == "c" else nf
                    nc.vector.tensor_reduce(
                        dst[:, p0:p0 + npt],
                        ps2[:, :npair].rearrange("p (n k) -> p n k", k=K_NN),
                        axis=AX.X, op=ALU.max)
            for j0 in range(0, NPT, CH):
                npt = min(CH, NPT - j0)
                ps3 = pp.tile([128, CH], F32, tag="mm")
                nc.tensor.matmul(ps3[:, :npt], WT["w3a"][:], cf[:, j0:j0 + npt],
                                 start=True, stop=False)
                nc.tensor.matmul(ps3[:, :npt], WT["w3b"][:], nf[:, j0:j0 + npt],
                                 start=False, stop=True)
                a3 = mp.tile([128, CH], BF16, tag="a3")
                nc.scalar.activation(a3[:, :npt], ps3[:, :npt], AF.Gelu)
                ps4 = pp.tile([128, CH], F32, tag="mm")
                nc.tensor.matmul(ps4[:, :npt], WT["w4"][:], a3[:, :npt],
                                 start=True, stop=True)
                o = mp.tile([128, CH], F32, tag="o")
                nc.scalar.copy(o[:, :npt], ps4[:, :npt])
                nc.sync.dma_start(out=enh_out[:, j0:j0 + npt], in_=o[:, :npt])
    nc.compile()
    return nc


# ================================================================ LAUNCH 2
_L2_WNAMES = []


def _l2_weight_decls():
    names = [("xa", [128, LSEQ], BF16), ("xb", [6, LSEQ], BF16),
             ("mask", [128, LSEQ], BF16),
             ("smask", [128, LSEQ], BF16),
             ("invn", [128, 1], F32),
             ("win_a", [128, 128], BF16), ("win_b", [6, 128], BF16)]
    for k in range(4):
        names += [(f"ln_g{k}", [128, 1], F32), (f"ln_b{k}", [128, 1], F32),
                  (f"wim{k}", [128, 256], BF16), (f"cw{k}", [128, 4], F32),
                  (f"cb{k}", [128, 1], F32), (f"xpw{k}", [128, 40], BF16),
                  (f"dtw{k}", [8, 128], BF16), (f"dtbn{k}", [128, 1], F32),
                  (f"dp2{k}", [128, 1], F32), (f"wout{k}", [128, 128], BF16),
                  (f"wob{k}", [128, 1], F32)]
    names += [("norm_g", [128, 1], F32), ("norm_b", [128, 1], F32),
              ("sel", [40, 32 * 128], BF16),
              ("pw1a", [128, 512], BF16), ("pw1b", [128, 512], BF16),
              ("pw2", [128, 2048], BF16)]
    return names


def _layernorm(nc, ap_, sm, pp, seq, xh, g, b, ones, eps, onesf):
    L = LSEQ
    mu = sm.tile([1, L], F32, tag="ln_mu")
    mu2 = sm.tile([1, L], F32, tag="ln_mu2")
    std = sm.tile([1, L], F32, tag="ln_std")
    sq = ap_.tile([128, L], F32, tag="cacc")          # alias cacc slot
    nc.scalar.activation(sq[:], seq[:], AF.Square)
    for c0, cw in CHS:
        sl = slice(c0, c0 + cw)
        pm = pp.tile([1, 512], F32, tag="mm", name="pm")
        nc.tensor.matmul(pm[:, :cw], ones[:], seq[:, sl], start=True, stop=True)
        nc.vector.tensor_copy(mu[:, sl], pm[:, :cw])
        pm2 = pp.tile([1, 512], F32, tag="mm", name="pm2")
        nc.tensor.matmul(pm2[:, :cw], ones[:], sq[:, sl], start=True, stop=True)
        nc.vector.tensor_copy(mu2[:, sl], pm2[:, :cw])
    nc.vector.tensor_mul(std[:], mu[:], mu[:])
    nc.vector.tensor_sub(mu2[:], mu2[:], std[:])
    nc.scalar.activation(std[:], mu2[:], AF.Sqrt, bias=eps[:])
    rstd = mu2
    nc.vector.reciprocal(rstd[:], std[:])
    for c0, cw in CHS:
        sl = slice(c0, c0 + cw)
        mub = pp.tile([128, 512], F32, tag="mm", name="mub")
        nc.tensor.matmul(mub[:, :cw], onesf[:], mu[:, sl], start=True, stop=True)
        nc.vector.tensor_sub(xh[:, sl], seq[:, sl], mub[:, :cw])
        rstdb = pp.tile([128, 512], F32, tag="mm", name="rstdb")
        nc.tensor.matmul(rstdb[:, :cw], onesf[:], rstd[:, sl], start=True, stop=True)
        nc.vector.tensor_mul(xh[:, sl], xh[:, sl], rstdb[:, :cw])
    nc.vector.tensor_scalar(out=xh[:], in0=xh[:], scalar1=g[:], scalar2=b[:],
                            op0=ALU.mult, op1=ALU.add)


def _mamba_block(nc, ap_, gp, sm, pp, bcp, WT, seq, ones, sel, k):
    L = LSEQ
    xh = ap_.tile([128, L], F32, tag="xh")
    _layernorm(nc, ap_, sm, pp, seq, xh, WT[f"ln_g{k}"], WT[f"ln_b{k}"], ones, WT["eps_"], WT["onesf_"])
    xh_bf = ap_.tile([128, L], BF16, tag="ybf")       # alias ybf slot
    nc.vector.tensor_copy(xh_bf[:], xh[:])
    xi = ap_.tile([128, L], BF16, tag="xi")
    sz = ap_.tile([128, L], BF16, tag="sz")
    for c0, cw in CHS:
        sl = slice(c0, c0 + cw)
        p1 = pp.tile([128, 512], F32, tag="mm")
        nc.tensor.matmul(p1[:, :cw], WT[f"wim{k}"][:, 0:128], xh_bf[:, sl],
                         start=True, stop=True)
        nc.scalar.copy(xi[:, sl], p1[:, :cw])
        p2 = pp.tile([128, 512], F32, tag="mm")
        nc.tensor.matmul(p2[:, :cw], WT[f"wim{k}"][:, 128:256], xh_bf[:, sl],
                         start=True, stop=True)
        nc.scalar.activation(sz[:, sl], p2[:, :cw], AF.Silu)
    cacc = ap_.tile([128, L], F32, tag="cacc")
    cw_t = WT[f"cw{k}"]
    nc.vector.tensor_scalar(out=cacc[:], in0=xi[:], scalar1=cw_t[:, 3:4],
                            scalar2=WT[f"cb{k}"][:], op0=ALU.mult, op1=ALU.add)
    for j in range(3):
        sh = 3 - j
        nc.vector.scalar_tensor_tensor(cacc[:, sh:], xi[:, :L - sh],
                                       cw_t[:, j:j + 1], cacc[:, sh:],
                                       ALU.mult, ALU.add)
    xic = ap_.tile([128, L], BF16, tag="xic")
    nc.scalar.activation(xic[:], cacc[:], AF.Silu)
    dbc_bf = ap_.tile([40, L], BF16, tag="dbc_bf")
    for c0, cw in CHS:
        sl = slice(c0, c0 + cw)
        p3 = pp.tile([40, 512], F32, tag="mm")
        nc.tensor.matmul(p3[:, :cw], WT[f"xpw{k}"][:], xic[:, sl],
                         start=True, stop=True)
        nc.scalar.copy(dbc_bf[:, sl], p3[:, :cw])
    # dt = softplus(p) via sigmoid/ln: dt_f holds ln(sigmoid(-p)) = -dt
    sig = ap_.tile([128, L], F32, tag="y", name="sig")
    dt_f = ap_.tile([128, L], F32, tag="dt_f")
    for c0, cw in CHS:
        sl = slice(c0, c0 + cw)
        p4 = pp.tile([128, 512], F32, tag="mm")
        nc.tensor.matmul(p4[:, :cw], WT[f"dtw{k}"][:], dbc_bf[0:8, sl],
                         start=True, stop=True)
        nc.scalar.activation(sig[:, sl], p4[:, :cw], AF.Sigmoid,
                             scale=-1.0, bias=WT[f"dtbn{k}"][:])
    nc.scalar.activation(dt_f[:], sig[:], AF.Ln)
    u = ap_.tile([128, L], BF16, tag="u")
    nc.vector.tensor_scalar_mul(u[:], dt_f[:], -1.0)
    nc.vector.tensor_mul(u[:], u[:], xic[:])
    nc.vector.tensor_mul(u[:], u[:], WT["smask"][:])
    y = ap_.tile([128, L], F32, tag="y")
    fl = lambda t: t.rearrange("p s l -> p (s l)")
    for g in range(S // SGRP):
        dA_f = gp.tile([128, SGRP, L], BF16, tag="dA_f", bufs=2)
        dBx = gp.tile([128, SGRP, L], BF16, tag="dBx", bufs=2)
        hf = gp.tile([128, SGRP, L], BF16, tag="hf")
        hb = gp.tile([128, SGRP, L], BF16, tag="dA_b", name="hb")
        Bb = gp.tile([128, SGRP, L], BF16, tag="Bb", bufs=2)
        Cb = gp.tile([128, SGRP, L], BF16, tag="Cb", bufs=2)
        for si in range(SGRP):
            s = g * SGRP + si
            nc.scalar.activation(dA_f[:, si, :], dt_f[:], AF.Exp,
                                 scale=float(s + 1))
            pb = bcp.tile([128, L], F32, tag="bc", name="pb")
            for c0, cw in CHS:
                nc.tensor.matmul(pb[:, c0:c0 + cw], sel[:, s * 128:(s + 1) * 128],
                                 dbc_bf[0:40, c0:c0 + cw], start=True, stop=True)
            nc.scalar.copy(Bb[:, si, :], pb[:])
            pc = bcp.tile([128, L], F32, tag="bc", name="pc")
            for c0, cw in CHS:
                nc.tensor.matmul(pc[:, c0:c0 + cw],
                                 sel[:, (16 + s) * 128:(17 + s) * 128],
                                 dbc_bf[0:40, c0:c0 + cw], start=True, stop=True)
            nc.scalar.copy(Cb[:, si, :], pc[:])
        # dBx is zero on every pad column (smask), and col L-1 is always a
        # pad column, so zeroing dA there makes the fwd carry self-killing
        # (h(seg end) = 0) and kills the bwd carry directly -- one dA tensor
        # serves both scan directions.
        nc.vector.memset(dA_f[:, :, L - 1:L], 0.0)
        nc.vector.tensor_mul(dBx[:], rep_ap(u[:], SGRP), Bb[:])
        nc.vector.tensor_tensor_scan(fl(hf[:]), fl(dA_f[:]), fl(dBx[:]),
                                     0.0, ALU.mult, ALU.add)
        nc.vector.tensor_tensor_scan(rev_ap(fl(hb[:])), rev_ap(fl(dA_f[:])),
                                     rev_ap(fl(dBx[:])), 0.0, ALU.mult, ALU.add)
        nc.gpsimd.tensor_add(hf[:], hf[:], hb[:])
        nc.vector.tensor_mul(hf[:], hf[:], Cb[:])
        if g == 0:
            nc.vector.tensor_add(y[:], hf[:, 0, :], hf[:, 1, :])
        else:
            nc.gpsimd.tensor_add(hf[:, 0, :], hf[:, 0, :], hf[:, 1, :])
            nc.vector.tensor_add(y[:], y[:], hf[:, 0, :])
    nc.vector.scalar_tensor_tensor(y[:], xic[:], WT[f"dp2{k}"][:], y[:],
                                   ALU.mult, ALU.add)
    ybf = ap_.tile([128, L], BF16, tag="ybf")
    nc.vector.tensor_mul(ybf[:], y[:], sz[:])
    for c0, cw in CHS:
        sl = slice(c0, c0 + cw)
        p5 = pp.tile([128, 512], F32, tag="mm")
        nc.tensor.matmul(p5[:, :cw], WT[f"wout{k}"][:], ybf[:, sl],
                         start=True, stop=True)
        nc.vector.scalar_tensor_tensor(seq[:, sl], p5[:, :cw], WT[f"wob{k}"][:],
                                       seq[:, sl], ALU.add, ALU.add)


def _build_l2():
    L = LSEQ
    nc = _mk_nc()
    decls = _l2_weight_decls()
    dp = {n: nc.declare_dram_parameter(n, s, d, isOutput=False)
          for n, s, d in decls}
    eo_out = nc.declare_dram_parameter("eo", [128, 4], F32, isOutput=True)

    with tile.TileContext(nc) as tc:
        with tc.tile_pool(name="w", bufs=1) as wp, \
             tc.tile_pool(name="act", bufs=1) as ap_, \
             tc.tile_pool(name="grp", bufs=1) as gp, \
             tc.tile_pool(name="sm", bufs=1) as sm, \
             tc.tile_pool(name="ps", bufs=3, space="PSUM") as pp, \
             tc.tile_pool(name="bc", bufs=1, space="PSUM") as bcp:
            WT = {}
            for n, s, d in decls:
                tag = n
                WT[n] = wp.tile(list(s), d, tag=tag, name="t_" + n)
                nc.sync.dma_start(out=WT[n][:], in_=dp[n][:])
            ones = wp.tile([128, 1], F32, tag="ones_")
            nc.vector.memset(ones[:], 1.0 / 128.0)
            eps_t = wp.tile([1, 1], F32, tag="eps_")
            nc.vector.memset(eps_t[:], 1e-5)
            WT["eps_"] = eps_t
            onesf = wp.tile([1, 128], F32, tag="onesf_")
            nc.vector.memset(onesf[:], 1.0)
            WT["onesf_"] = onesf
            sel = WT["sel"]

            seq = wp.tile([128, L], F32, tag="seq")
            for c0, cw in CHS:
                sl = slice(c0, c0 + cw)
                pst = pp.tile([128, 512], F32, tag="mm")
                nc.tensor.matmul(pst[:, :cw], WT["win_a"][:], WT["xa"][:, sl],
                                 start=True, stop=False)
                nc.tensor.matmul(pst[:, :cw], WT["win_b"][:], WT["xb"][:, sl],
                                 start=False, stop=True)
                nc.scalar.copy(seq[:, sl], pst[:, :cw])

            for k in range(4):
                _mamba_block(nc, ap_, gp, sm, pp, bcp, WT, seq, ones, sel, k)

            xh = ap_.tile([128, L], F32, tag="xh")
            _layernorm(nc, ap_, sm, pp, seq, xh, WT["norm_g"], WT["norm_b"], ones, WT["eps_"], WT["onesf_"])
            nmt = ap_.tile([128, L], F32, tag="cacc", name="nmt")
            nc.vector.tensor_scalar(out=nmt[:], in0=WT["mask"][:], scalar1=1e9,
                                    scalar2=-1e9, op0=ALU.mult, op1=ALU.add)
            xm = ap_.tile([128, L], F32, tag="y")
            nc.vector.tensor_add(xm[:], xh[:], nmt[:])
            pmax = sm.tile([128, 1], F32, tag="pmax")
            nc.vector.tensor_reduce(pmax[:], xm[:], axis=AX.X, op=ALU.max)
            xs = ap_.tile([128, L], F32, tag="cacc")
            nc.vector.tensor_mul(xs[:], xh[:], WT["mask"][:])
            psm = sm.tile([128, 1], F32, tag="psm")
            nc.vector.tensor_reduce(psm[:], xs[:], axis=AX.X, op=ALU.add)
            pmean = sm.tile([128, 1], F32, tag="pmean")
            nc.vector.tensor_mul(pmean[:], psm[:], WT["invn"][:])
            pool_bf = sm.tile([128, 2], BF16, tag="pool_bf")
            nc.vector.tensor_copy(pool_bf[:, 0:1], pmax[:])
            nc.vector.tensor_copy(pool_bf[:, 1:2], pmean[:])
            ps_m = pp.tile([128, 4], F32, tag="mm")
            for m in range(4):
                nc.tensor.matmul(ps_m[:, m:m + 1],
                                 WT["pw1a"][:, m * 128:(m + 1) * 128],
                                 pool_bf[:, 0:1], start=True, stop=False)
                nc.tensor.matmul(ps_m[:, m:m + 1],
                                 WT["pw1b"][:, m * 128:(m + 1) * 128],
                                 pool_bf[:, 1:2], start=False, stop=True)
            h1 = sm.tile([128, 4], BF16, tag="h1")
            nc.scalar.activation(h1[:], ps_m[:], AF.Gelu)
            ps_o = pp.tile([128, 4], F32, tag="mm")
            for m in range(4):
                for kc in range(4):
                    nc.tensor.matmul(
                        ps_o[:, m:m + 1],
                        WT["pw2"][:, kc * 512 + m * 128:kc * 512 + (m + 1) * 128],
                        h1[:, kc:kc + 1], start=(kc == 0), stop=(kc == 3))
            eo_t = sm.tile([128, 4], F32, tag="eo_t")
            nc.scalar.copy(eo_t[:], ps_o[:])
            nc.sync.dma_start(out=eo_out[:], in_=eo_t[:])
    nc.compile()
    return nc


# ================================================================ LAUNCH 3
def _build_l3():
    nc = _mk_nc()
    din = {}
    def D(name, shape, dt=BF16):
        din[name] = nc.declare_dram_parameter(name, shape, dt, isOutput=False)
    D("gfT", [128, 24])
    D("gw1", [128, 3 * 12 * 512])
    D("gw2", [128, 3 * 4 * 512])
    D("dwl", [128, 4 * CPC_L]); D("dwm", [128, 4 * CPC_M]); D("dwh", [128, 4 * CPC_H])
    outs = {"ol": nc.declare_dram_parameter("ol", [CPC_L, 2], F32, isOutput=True),
            "om": nc.declare_dram_parameter("om", [CPC_M, 2], F32, isOutput=True),
            "oh": nc.declare_dram_parameter("oh", [CPC_H, 2], F32, isOutput=True)}

    with tile.TileContext(nc) as tc:
        with tc.tile_pool(name="w", bufs=1) as wp, \
             tc.tile_pool(name="sm", bufs=1) as sm, \
             tc.tile_pool(name="ps", bufs=2, space="PSUM") as pp:
            WT = {}
            for name, t in din.items():
                WT[name] = wp.tile(list(t.shape), t.dtype, tag=name, name="t_" + name)
                nc.sync.dma_start(out=WT[name][:], in_=t[:])
            f2s = []
            for i in range(3):
                psf = pp.tile([128, 4, 2], F32, tag="mm")
                for m in range(4):
                    for kc in range(12):
                        w = WT["gw1"][:, ((i * 12 + kc) * 4 + m) * 128:
                                      ((i * 12 + kc) * 4 + m + 1) * 128]
                        nc.tensor.matmul(psf[:, m, :], w,
                                         WT["gfT"][:, kc * 2:(kc + 1) * 2],
                                         start=(kc == 0), stop=(kc == 11))
                f_bf = wp.tile([128, 4, 2], BF16, tag=f"f_bf{i}")
                nc.scalar.activation(f_bf[:], psf[:], AF.Gelu)
                ps2 = pp.tile([128, 4, 2], F32, tag="mm")
                for m in range(4):
                    for kc in range(4):
                        w = WT["gw2"][:, ((i * 4 + kc) * 4 + m) * 128:
                                      ((i * 4 + kc) * 4 + m + 1) * 128]
                        nc.tensor.matmul(ps2[:, m, :], w, f_bf[:, kc, :],
                                         start=(kc == 0), stop=(kc == 3))
                f2 = wp.tile([128, 4, 2], BF16, tag=f"f2_{i}")
                nc.vector.tensor_copy(f2[:], ps2[:])
                f2s.append(f2)
            # dec: low <- f3 (i=2), mid <- f2 (i=1), high <- f1 (i=0)
            for nm, cpc, fi in (("l", CPC_L, 2), ("m", CPC_M, 1), ("h", CPC_H, 0)):
                mcs = [(m0, min(128, cpc - m0)) for m0 in range(0, cpc, 128)]
                for m0, mw in mcs:
                    psd = pp.tile([128, 2], F32, tag="mm")
                    for kc in range(4):
                        w = WT["dw" + nm][:, kc * cpc + m0: kc * cpc + m0 + mw]
                        nc.tensor.matmul(psd[:mw, :], w, f2s[fi][:, kc, :],
                                         start=(kc == 0), stop=(kc == 3))
                    ot = sm.tile([128, 2], F32, tag="ot")
                    nc.scalar.copy(ot[:mw, :], psd[:mw, :])
                    nc.sync.dma_start(out=outs["o" + nm][m0:m0 + mw, :],
                                      in_=ot[:mw, :])
    nc.compile()
    return nc


# ================================================================ host packing
def _prep_unit(sampled_b, enhT_b, order_b, win):
    """Build xa/xb/mask/nmask/invn for one (b,res) unit."""
    n = sampled_b.shape[0]
    xin = np.zeros((134, LSEQ), np.float32)
    xin[0:6, 1:n + 1] = sampled_b[order_b].T
    xin[6:134, 1:n + 1] = enhT_b
    # oip_e column: v solving win.T @ v = oip_e
    return xin


def _pack_l2_core(xin, v_e, n, enc_p):
    L = LSEQ
    xin = xin.copy()
    xin[:, n + 1] = v_e
    mask = np.zeros((128, L), np.float32); mask[:, 1:n + 1] = 1.0
    smask = np.zeros((128, L), np.float32); smask[:, 0:n + 2] = 1.0

    m = {"xa": _bf(xin[0:128]), "xb": _bf(xin[128:134]),
         "mask": _bf(mask), "smask": _bf(smask),
         "invn": _f32(np.full((128, 1), 1.0 / n))}
    w_in = _f32(enc_p['in']['w'])
    m["win_a"] = _bf(w_in[0:128]); m["win_b"] = _bf(w_in[128:134])
    for k, blk in enumerate(enc_p['blocks']):
        m[f"ln_g{k}"] = _col(blk['ln_g']); m[f"ln_b{k}"] = _col(blk['ln_b'])
        m[f"wim{k}"] = _bf(blk['in']['w'])
        m[f"cw{k}"] = _f32(blk['conv_w']); m[f"cb{k}"] = _col(blk['conv_b'])
        m[f"xpw{k}"] = _bf(blk['xp_w'])
        m[f"dtw{k}"] = _bf(blk['dt']['w'])       # [R=8, di=128] is lhsT already
        m[f"dtbn{k}"] = _col(-np.asarray(blk['dt']['b'], np.float32))
        m[f"dp2{k}"] = _col(2.0 * np.asarray(blk['Dp'], np.float32))
        m[f"wout{k}"] = _bf(blk['out']['w']); m[f"wob{k}"] = _col(blk['out']['b'])
    m["norm_g"] = _col(enc_p['norm_g']); m["norm_b"] = _col(enc_p['norm_b'])
    seln = np.zeros((40, 32 * 128), np.float32)
    for idx in range(32):
        seln[8 + idx, idx * 128:(idx + 1) * 128] = 1.0
    m["sel"] = _bf(seln)
    pw1 = _f32(enc_p['out']['l1']['w'])
    m["pw1a"] = _bf(pw1[0:128]); m["pw1b"] = _bf(pw1[128:256])
    pw2 = _f32(enc_p['out']['l2']['w'])
    m["pw2"] = _bf(np.concatenate([pw2[kc * 128:(kc + 1) * 128, :]
                                   for kc in range(4)], axis=1))
    return m


def _dummy_l2_core():
    z = {n: np.zeros(s, BF if d == BF16 else np.float32)
         for n, s, d in _l2_weight_decls()}
    z["invn"][:] = 1.0
    return z


def kernel(arch_points, arch_blocks, params):
    params = _np_tree(params)
    arch_blocks = np.asarray(arch_blocks, np.float32)

    res_cfg = [("low", LOW), ("mid", MID), ("high", HIGH)]
    units = []           # list of dicts in order (res, b)
    pair_feats = []      # [7, n*32] per unit
    for rname, total in res_cfg:
        per = max(1, total // T)
        sampled = _fps(arch_blocks, per)            # [B, N, 6]
        xyz, nrm = sampled[..., :3], sampled[..., 3:6]
        knn = _knn(xyz)
        order = _zorder(xyz)
        for b in range(B):
            o = order[b]
            nb = knn[b][o]                           # [N, 32] (z-ordered rows)
            rel = xyz[b][nb] - xyz[b][o][:, None, :]
            nnb = nrm[b][nb]
            var = np.abs(1.0 - np.sum(nrm[b][o][:, None, :] * nnb, -1))
            pf = np.concatenate([rel.reshape(-1, 3),
                                 nnb.reshape(-1, 3),
                                 var.reshape(-1, 1)], axis=1).T  # [7, N*32]
            pair_feats.append(pf.astype(np.float32))
            units.append({"res": rname, "b": b, "n": sampled.shape[1],
                          "sampled": sampled[b], "order": o})

    # ---- launch 1
    allpairs = np.concatenate(pair_feats, axis=1)    # [7, NPTS*32]
    run1 = _runner("l1", _build_l1)
    enh_p = params['enh']
    w1 = {"w1c": _bf(enh_p['coord']['l1']['w']),
          "w1n": _bf(enh_p['normal']['l1']['w']),
          "w2c": _bf(enh_p['coord']['l2']['w']),
          "w2n": _bf(enh_p['normal']['l2']['w']),
          "w3a": _bf(enh_p['out']['l1']['w'][0:128]),
          "w3b": _bf(enh_p['out']['l1']['w'][128:256]),
          "w4": _bf(enh_p['out']['l2']['w'])}
    in_maps1 = []
    for c in range(NCORES):
        sl = slice(c * PAIRS_PER_CORE, (c + 1) * PAIRS_PER_CORE)
        pr = np.zeros((36, PAIRS_PER_CORE), np.float32)
        pr[0:3] = allpairs[0:3, sl]
        pr[32:36] = allpairs[3:7, sl]
        in_maps1.append({"pr": _bf(pr), **w1})
    res1 = run1(in_maps1)
    enhT = np.concatenate([res1[c]["enh"] for c in range(NCORES)], axis=1)  # [128, NPTS]

    # ---- launch 2
    run2 = _runner("l2", _build_l2)
    in_maps2 = []
    off = 0
    enc_key = {"low": "enc_low", "mid": "enc_mid", "high": "enc_high"}
    for u in units:
        n = u["n"]
        enc_p = params[enc_key[u["res"]]]
        xin = _prep_unit(u["sampled"], enhT[:, off:off + n], u["order"],
                         enc_p['in']['w'])
        off += n
        oip_e = (np.asarray(enc_p['oip']['w'], np.float32)[0]
                 + np.asarray(enc_p['oip']['b'], np.float32))
        w_in = _f32(enc_p['in']['w'])
        v_e = np.linalg.lstsq(w_in.T, oip_e - _f32(enc_p['in']['b']),
                              rcond=None)[0]
        in_maps2.append(_pack_l2_core(xin, v_e, n, enc_p))
    while len(in_maps2) < NCORES:
        in_maps2.append(_dummy_l2_core())
    res2 = run2(in_maps2)
    eo = [res2[c]["eo"].T.flatten() for c in range(6)]   # [512] each
    # units order: low_b0, low_b1, mid_b0, mid_b1, high_b0, high_b1
    gf = np.stack([np.concatenate([eo[0], eo[2], eo[4]]),
                   np.concatenate([eo[1], eo[3], eo[5]])], axis=0)  # [2,1536]

    # ---- launch 3
    run3 = _runner("l3", _build_l3)
    gfT = np.zeros((128, 24), np.float32)
    for kc in range(12):
        gfT[:, kc * 2:(kc + 1) * 2] = gf[:, kc * 128:(kc + 1) * 128].T
    gw1 = np.zeros((128, 3 * 12 * 512), np.float32)
    for i, gk in enumerate(("g1", "g2", "g3")):
        W1 = _f32(params[gk]['l1']['w'])
        for kc in range(12):
            for mch in range(4):
                gw1[:, ((i * 12 + kc) * 4 + mch) * 128:((i * 12 + kc) * 4 + mch + 1) * 128] = \
                    W1[kc * 128:(kc + 1) * 128, mch * 128:(mch + 1) * 128]
    gw2 = np.zeros((128, 3 * 4 * 512), np.float32)
    for i, gk in enumerate(("g1", "g2", "g3")):
        W2 = _f32(params[gk]['l2']['w'])
        for kc in range(4):
            for mch in range(4):
                gw2[:, ((i * 4 + kc) * 4 + mch) * 128:((i * 4 + kc) * 4 + mch + 1) * 128] = \
                    W2[kc * 128:(kc + 1) * 128, mch * 128:(mch + 1) * 128]
    in_maps3 = []
    dec = params['dec']
    for c in range(NCORES):
        m = {"gfT": _bf(gfT), "gw1": _bf(gw1), "gw2": _bf(gw2)}
        for nm, key, cpc in (("dwl", "low", CPC_L), ("dwm", "mid", CPC_M),
                             ("dwh", "high", CPC_H)):
            W = _f32(dec[key]['w'])                  # [512, LOWx3]
            sl = W[:, c * cpc:(c + 1) * cpc]
            m[nm] = _bf(np.concatenate([sl[kc * 128:(kc + 1) * 128, :]
                                        for kc in range(4)], axis=1))
        in_maps3.append(m)
    res3 = run3(in_maps3)
    dlo = np.concatenate([res3[c]["ol"].T for c in range(NCORES)], axis=1)
    dmd = np.concatenate([res3[c]["om"].T for c in range(NCORES)], axis=1)
    dhi = np.concatenate([res3[c]["oh"].T for c in range(NCORES)], axis=1)

    pl = dlo.reshape(B, LOW, 3)
    pm = _expand(pl, MID) + dmd.reshape(B, MID, 3)
    ph = _expand(pm, HIGH) + dhi.reshape(B, HIGH, 3)
    return (pl.astype(np.float32), pm.astype(np.float32),
            ph.astype(np.float32), gf.astype(np.float32))


# revision 24
# speedup vs baseline: 1.4530x; 1.1333x over previous
"""CMDenNet Trainium2 kernel: host prep (FPS/kNN/z-order/gathers) + 3 SPMD Bass launches.

L1: enhancer MLPs, 8-way row split (896 points x 32 neighbors per core).
L2: encoders (4 bidirectional-Mamba blocks + pooling + out MLP); one unit per
    core, zero-padded to a uniform L=2052 so a single static program serves all
    cores (6 real units + 2 dummy cores).
L3: heads (g1/g2/g3 MLPs replicated, dec projections column-split 8 ways).

All data-dependent indexing (FPS, kNN, Morton order) is computed on host and
folded into the input packing; device kernels are shape-static, so programs
compile once per process and are reused across calls.
"""
import sys
import numpy as np

sys.path.insert(0, '/opt/trn_rl_repo')

import ml_dtypes
import concourse.bass as bass
import concourse.tile as tile
from concourse import bacc, mybir
from concourse import bass2jax

F32 = mybir.dt.float32
BF16 = mybir.dt.bfloat16
AF = mybir.ActivationFunctionType
ALU = mybir.AluOpType
AX = mybir.AxisListType
BF = ml_dtypes.bfloat16

H = 128; G = 512; K_NN = 32; RES = 64; S = 16; R = 8; KCONV = 4
LOW, MID, HIGH = 512, 1024, 2048
B, T, P, PD = 2, 16, 256, 6
NCORES = 8
LSEQ = 2052
NPTS = (LOW + MID + HIGH) * B          # 7168
PTS_PER_CORE = NPTS // NCORES          # 896
PAIRS_PER_CORE = PTS_PER_CORE * K_NN   # 28672
SGRP = 2                               # s-states per SSM group
CHS = [(0, 512), (512, 512), (1024, 512), (1536, 512), (2048, 4)]
CPC_L, CPC_M, CPC_H = LOW * 3 // NCORES, MID * 3 // NCORES, HIGH * 3 // NCORES


# ---------------------------------------------------------------- host math
def _fps(blocks, n):
    b, t, pp, d = blocks.shape
    pts = blocks.reshape(b * t, pp, d)
    xyz = pts[..., :3]
    mind = np.sum((xyz - xyz[:, :1]) ** 2, -1)
    idxs = [np.zeros((b * t,), np.int32)]
    for _ in range(n - 1):
        nxt = np.argmax(mind, axis=1)
        sel = np.take_along_axis(xyz, nxt[:, None, None], axis=1)
        mind = np.minimum(mind, np.sum((xyz - sel) ** 2, -1))
        idxs.append(nxt.astype(np.int32))
    idx = np.stack(idxs, 0).T
    out = np.take_along_axis(pts, idx[..., None], axis=1)
    return out.reshape(b, t * n, d)


def _knn(xyz):
    bsz, n, _ = xyz.shape
    sq = np.sum(xyz * xyz, -1)
    out = np.empty((bsz, n, K_NN), np.int64)
    for bi in range(bsz):
        d2 = sq[bi][:, None] + sq[bi][None, :] - 2.0 * (xyz[bi] @ xyz[bi].T)
        idx = np.argsort(d2, axis=1, kind='stable')[:, :K_NN + 1]
        out[bi] = idx[:, 1:]
    return out


def _zorder(xyz):
    mn = xyz.min(axis=1, keepdims=True)
    mx = xyz.max(axis=1, keepdims=True)
    g = np.clip(((xyz - mn) / (mx - mn + 1e-9) * RES).astype(np.int32), 0, RES - 1)
    code = np.zeros(g.shape[:2], np.int32)
    for bit in range(6):
        for ax in range(3):
            code = code | (((g[..., ax] >> bit) & 1) << (3 * bit + ax))
    return np.argsort(code, axis=1, kind='stable')


def _expand(pts, out_n):
    rep = -(-out_n // pts.shape[1])
    return np.repeat(pts, rep, axis=1)[:, :out_n]


def _np_tree(tree):
    if isinstance(tree, dict):
        return {k: _np_tree(v) for k, v in tree.items()}
    if isinstance(tree, (list, tuple)):
        return [_np_tree(v) for v in tree]
    return np.asarray(tree)


def _bf(x):
    return np.ascontiguousarray(np.asarray(x, np.float32).astype(BF))


def _f32(x):
    return np.ascontiguousarray(np.asarray(x, np.float32))


def _col(x):
    return _f32(np.asarray(x).reshape(128, 1))


# ---------------------------------------------------------------- bass helpers
def _mk_nc():
    return bacc.Bacc("TRN2", target_bir_lowering=False, debug=False,
                     num_devices=NCORES)


def rev_ap(ap_in):
    (pstep, pcnt), (estep, ecnt) = ap_in.ap
    return bass.AP(tensor=ap_in.tensor, offset=ap_in.offset + (ecnt - 1) * estep,
                   ap=[[pstep, pcnt], [-estep, ecnt]])


def rep_ap(ap_in, n):
    """[P, L] viewed as [P, n, L] with the middle axis stride 0."""
    (pstep, pcnt), (estep, ecnt) = ap_in.ap
    return bass.AP(tensor=ap_in.tensor, offset=ap_in.offset,
                   ap=[[pstep, pcnt], [0, n], [estep, ecnt]])


_RUNNERS = {}


def _runner(key, build_fn):
    if key in _RUNNERS:
        return _RUNNERS[key]
    import jax
    from jax.sharding import Mesh, PartitionSpec
    from jax.experimental.shard_map import shard_map

    nc = build_fn()
    bass2jax.install_neuronx_cc_hook()
    partition_name = nc.partition_id_tensor.name if nc.partition_id_tensor else None
    in_names, out_names, out_avals, zero_shapes = [], [], [], []
    for alloc in nc.m.functions[0].allocations:
        if not isinstance(alloc, mybir.MemoryLocationSet):
            continue
        name = alloc.memorylocations[0].name
        if alloc.kind == "ExternalInput":
            if name != partition_name:
                in_names.append(name)
        elif alloc.kind == "ExternalOutput":
            out_names.append(name)
            shape = tuple(alloc.tensor_shape)
            dtype = mybir.dt.np(alloc.dtype)
            out_avals.append(jax.core.ShapedArray(shape, dtype))
            zero_shapes.append((shape, dtype))
    n_params = len(in_names)
    n_outs = len(out_avals)
    in_names_all = list(in_names) + list(out_names)
    if partition_name is not None:
        in_names_all.append(partition_name)

    def _body(*args):
        operands = list(args)
        if partition_name is not None:
            operands.append(bass2jax.partition_id_tensor())
        outs = bass2jax._bass_exec_p.bind(
            *operands, out_avals=tuple(out_avals), in_names=tuple(in_names_all),
            out_names=tuple(out_names), lowering_input_output_aliases=(),
            sim_require_finite=False, sim_require_nnan=False, nc=nc)
        return tuple(outs)

    donate = tuple(range(n_params, n_params + n_outs))
    devices = jax.devices()[:NCORES]
    mesh = Mesh(np.asarray(devices), ("core",))
    jfn = jax.jit(shard_map(_body, mesh=mesh,
                            in_specs=(PartitionSpec("core"),) * (n_params + n_outs),
                            out_specs=(PartitionSpec("core"),) * n_outs,
                            check_rep=False),
                  donate_argnums=donate, keep_unused=True)

    def run(in_maps):
        per_core = [[np.ascontiguousarray(m[n]) for n in in_names] for m in in_maps]
        concat_in = [np.concatenate([per_core[c][i] for c in range(NCORES)], axis=0)
                     for i in range(n_params)]
        concat_zero = [np.zeros((NCORES * s[0], *s[1:]), d) for s, d in zero_shapes]
        outs = jfn(*concat_in, *concat_zero)
        outs = [np.asarray(o) for o in outs]
        return [{name: outs[i].reshape(NCORES, *out_avals[i].shape)[c]
                 for i, name in enumerate(out_names)} for c in range(NCORES)]

    _RUNNERS[key] = run
    return run


# ================================================================ LAUNCH 1
def _build_l1():
    NPAIR, NPT, CH = PAIRS_PER_CORE, PTS_PER_CORE, 512
    nc = _mk_nc()
    pr_in = nc.declare_dram_parameter("pr", [36, NPAIR], BF16, isOutput=False)
    names = [("w1c", [3, 128]), ("w1n", [4, 128]), ("w2c", [128, 128]),
             ("w2n", [128, 128]), ("w3a", [128, 128]), ("w3b", [128, 128]),
             ("w4", [128, 128])]
    dins = {n: nc.declare_dram_parameter(n, s, BF16, isOutput=False) for n, s in names}
    enh_out = nc.declare_dram_parameter("enh", [128, NPT], F32, isOutput=True)

    with tile.TileContext(nc) as tc:
        with tc.tile_pool(name="w", bufs=1) as wp, \
             tc.tile_pool(name="mid", bufs=3) as mp, \
             tc.tile_pool(name="ps", bufs=3, space="PSUM") as pp:
            WT = {}
            for n, s in names:
                if n == "w1n":
                    w1n_t = wp.tile([36, 128], BF16, tag=n, name="t_" + n)
                    nc.sync.dma_start(out=w1n_t[32:36, :], in_=dins[n][:])
                    WT[n] = w1n_t
                else:
                    WT[n] = wp.tile(s, BF16, tag=n, name="t_" + n)
                    nc.sync.dma_start(out=WT[n][:], in_=dins[n][:])
            pr = wp.tile([36, NPAIR], BF16, tag="pr")
            nc.sync.dma_start(out=pr[:], in_=pr_in[:])
            cf = wp.tile([128, NPT], BF16, tag="cf")
            nf = wp.tile([128, NPT], BF16, tag="nf")

            for j0 in range(0, NPAIR, CH):
                npair = min(CH, NPAIR - j0)
                npt = npair // K_NN
                p0 = j0 // K_NN
                for bk in ("c", "n"):
                    rows = pr[0:3, j0:j0 + npair] if bk == "c" else pr[32:36, j0:j0 + npair]
                    w1 = WT["w1c"][:] if bk == "c" else WT["w1n"][32:36, :]
                    ps1 = pp.tile([128, CH], F32, tag="mm")
                    nc.tensor.matmul(ps1[:, :npair], w1, rows,
                                     start=True, stop=True)
                    a1 = mp.tile([128, CH], BF16, tag="a1")
                    nc.scalar.activation(a1[:, :npair], ps1[:, :npair], AF.Gelu)
                    ps2 = pp.tile([128, CH], F32, tag="mm")
                    nc.tensor.matmul(ps2[:, :npair], WT["w2" + bk][:], a1[:, :npair],
                                     start=True, stop=True)
                    dst = cf if bk == "c" else nf
                    nc.vector.tensor_reduce(
                        dst[:, p0:p0 + npt],
                        ps2[:, :npair].rearrange("p (n k) -> p n k", k=K_NN),
                        axis=AX.X, op=ALU.max)
            for j0 in range(0, NPT, CH):
                npt = min(CH, NPT - j0)
                ps3 = pp.tile([128, CH], F32, tag="mm")
                nc.tensor.matmul(ps3[:, :npt], WT["w3a"][:], cf[:, j0:j0 + npt],
                                 start=True, stop=False)
                nc.tensor.matmul(ps3[:, :npt], WT["w3b"][:], nf[:, j0:j0 + npt],
                                 start=False, stop=True)
                a3 = mp.tile([128, CH], BF16, tag="a3")
                nc.scalar.activation(a3[:, :npt], ps3[:, :npt], AF.Gelu)
                ps4 = pp.tile([128, CH], F32, tag="mm")
                nc.tensor.matmul(ps4[:, :npt], WT["w4"][:], a3[:, :npt],
                                 start=True, stop=True)
                o = mp.tile([128, CH], F32, tag="o")
                nc.scalar.copy(o[:, :npt], ps4[:, :npt])
                nc.sync.dma_start(out=enh_out[:, j0:j0 + npt], in_=o[:, :npt])
    nc.compile()
    return nc


# ================================================================ LAUNCH 2
_L2_WNAMES = []


def _l2_weight_decls():
    names = [("xa", [128, LSEQ], BF16), ("xb", [6, LSEQ], BF16),
             ("mask", [128, LSEQ], BF16),
             ("smask", [128, LSEQ], BF16),
             ("invn", [128, 1], F32),
             ("win_a", [128, 128], BF16), ("win_b", [6, 128], BF16)]
    for k in range(4):
        names += [(f"ln_g{k}", [128, 1], F32), (f"ln_b{k}", [128, 1], F32),
                  (f"wim{k}", [128, 256], BF16), (f"cw{k}", [128, 4], F32),
                  (f"cb{k}", [128, 1], F32), (f"xpw{k}", [128, 40], BF16),
                  (f"dtw{k}", [8, 128], BF16), (f"dtbn{k}", [128, 1], F32),
                  (f"dp2{k}", [128, 1], F32), (f"wout{k}", [128, 128], BF16),
                  (f"wob{k}", [128, 1], F32)]
    names += [("norm_g", [128, 1], F32), ("norm_b", [128, 1], F32),
              ("sel", [40, 32 * 128], BF16),
              ("pw1a", [128, 512], BF16), ("pw1b", [128, 512], BF16),
              ("pw2", [128, 2048], BF16)]
    return names


def _layernorm(nc, ap_, sm, pp, seq, xh, g, b, ones, eps, onesf):
    L = LSEQ
    mu = sm.tile([1, L], F32, tag="ln_mu")
    mu2 = sm.tile([1, L], F32, tag="ln_mu2")
    std = sm.tile([1, L], F32, tag="ln_std")
    sq = ap_.tile([128, L], F32, tag="cacc")          # alias cacc slot
    nc.scalar.activation(sq[:], seq[:], AF.Square)
    for c0, cw in CHS:
        sl = slice(c0, c0 + cw)
        pm = pp.tile([1, 512], F32, tag="mm", name="pm")
        nc.tensor.matmul(pm[:, :cw], ones[:], seq[:, sl], start=True, stop=True)
        nc.vector.tensor_copy(mu[:, sl], pm[:, :cw])
        pm2 = pp.tile([1, 512], F32, tag="mm", name="pm2")
        nc.tensor.matmul(pm2[:, :cw], ones[:], sq[:, sl], start=True, stop=True)
        nc.vector.tensor_copy(mu2[:, sl], pm2[:, :cw])
    nc.vector.tensor_mul(std[:], mu[:], mu[:])
    nc.vector.tensor_sub(mu2[:], mu2[:], std[:])
    nc.scalar.activation(std[:], mu2[:], AF.Sqrt, bias=eps[:])
    rstd = mu2
    nc.vector.reciprocal(rstd[:], std[:])
    for c0, cw in CHS:
        sl = slice(c0, c0 + cw)
        mub = pp.tile([128, 512], F32, tag="mm", name="mub")
        nc.tensor.matmul(mub[:, :cw], onesf[:], mu[:, sl], start=True, stop=True)
        nc.vector.tensor_sub(xh[:, sl], seq[:, sl], mub[:, :cw])
        rstdb = pp.tile([128, 512], F32, tag="mm", name="rstdb")
        nc.tensor.matmul(rstdb[:, :cw], onesf[:], rstd[:, sl], start=True, stop=True)
        nc.vector.tensor_mul(xh[:, sl], xh[:, sl], rstdb[:, :cw])
    nc.vector.tensor_scalar(out=xh[:], in0=xh[:], scalar1=g[:], scalar2=b[:],
                            op0=ALU.mult, op1=ALU.add)


def _mamba_block(nc, ap_, gp, sm, pp, bcp, WT, seq, ones, sel, k):
    L = LSEQ
    xh = ap_.tile([128, L], F32, tag="xh")
    _layernorm(nc, ap_, sm, pp, seq, xh, WT[f"ln_g{k}"], WT[f"ln_b{k}"], ones, WT["eps_"], WT["onesf_"])
    xh_bf = ap_.tile([128, L], BF16, tag="ybf")       # alias ybf slot
    for c0, cw in CHS:
        nc.vector.tensor_copy(xh_bf[:, c0:c0 + cw], xh[:, c0:c0 + cw])
    xi = ap_.tile([128, L], BF16, tag="xi")
    sz = ap_.tile([128, L], BF16, tag="sz")
    for c0, cw in CHS:
        sl = slice(c0, c0 + cw)
        p1 = pp.tile([128, 512], F32, tag="mm")
        nc.tensor.matmul(p1[:, :cw], WT[f"wim{k}"][:, 0:128], xh_bf[:, sl],
                         start=True, stop=True)
        nc.scalar.copy(xi[:, sl], p1[:, :cw])
        p2 = pp.tile([128, 512], F32, tag="mm")
        nc.tensor.matmul(p2[:, :cw], WT[f"wim{k}"][:, 128:256], xh_bf[:, sl],
                         start=True, stop=True)
        nc.scalar.activation(sz[:, sl], p2[:, :cw], AF.Silu)
    cacc = ap_.tile([128, L], F32, tag="cacc")
    cw_t = WT[f"cw{k}"]
    nc.vector.tensor_scalar(out=cacc[:], in0=xi[:], scalar1=cw_t[:, 3:4],
                            scalar2=WT[f"cb{k}"][:], op0=ALU.mult, op1=ALU.add)
    for j in range(3):
        sh = 3 - j
        nc.vector.scalar_tensor_tensor(cacc[:, sh:], xi[:, :L - sh],
                                       cw_t[:, j:j + 1], cacc[:, sh:],
                                       ALU.mult, ALU.add)
    xic = ap_.tile([128, L], BF16, tag="xic")
    nc.scalar.activation(xic[:], cacc[:], AF.Silu)
    dbc_bf = ap_.tile([40, L], BF16, tag="dbc_bf")
    for c0, cw in CHS:
        sl = slice(c0, c0 + cw)
        p3 = pp.tile([40, 512], F32, tag="mm")
        nc.tensor.matmul(p3[:, :cw], WT[f"xpw{k}"][:], xic[:, sl],
                         start=True, stop=True)
        nc.scalar.copy(dbc_bf[:, sl], p3[:, :cw])
    # dt = softplus(p) via sigmoid/ln: dt_f holds ln(sigmoid(-p)) = -dt
    sig = ap_.tile([128, L], F32, tag="y", name="sig")
    dt_f = ap_.tile([128, L], F32, tag="dt_f")
    for c0, cw in CHS:
        sl = slice(c0, c0 + cw)
        p4 = pp.tile([128, 512], F32, tag="mm")
        nc.tensor.matmul(p4[:, :cw], WT[f"dtw{k}"][:], dbc_bf[0:8, sl],
                         start=True, stop=True)
        nc.scalar.activation(sig[:, sl], p4[:, :cw], AF.Sigmoid,
                             scale=-1.0, bias=WT[f"dtbn{k}"][:])
    nc.scalar.activation(dt_f[:], sig[:], AF.Ln)
    u = ap_.tile([128, L], BF16, tag="u")
    nc.vector.tensor_scalar_mul(u[:], dt_f[:], -1.0)
    nc.vector.tensor_mul(u[:], u[:], xic[:])
    nc.vector.tensor_mul(u[:], u[:], WT["smask"][:])
    y = ap_.tile([128, L], F32, tag="y")
    fl = lambda t: t.rearrange("p s l -> p (s l)")
    for g in range(S // SGRP):
        dA_f = gp.tile([128, SGRP, L], BF16, tag="dA_f", bufs=2)
        dBx = gp.tile([128, SGRP, L], BF16, tag="dBx", bufs=2)
        hf = gp.tile([128, SGRP, L], BF16, tag="hf")
        hb = gp.tile([128, SGRP, L], BF16, tag="dA_b", name="hb")
        Bb = gp.tile([128, SGRP, L], BF16, tag="Bb", bufs=2)
        Cb = gp.tile([128, SGRP, L], BF16, tag="Cb", bufs=2)
        for si in range(SGRP):
            s = g * SGRP + si
            nc.scalar.activation(dA_f[:, si, :], dt_f[:], AF.Exp,
                                 scale=float(s + 1))
            pb = bcp.tile([128, L], F32, tag="bc", name="pb")
            for c0, cw in CHS:
                nc.tensor.matmul(pb[:, c0:c0 + cw], sel[:, s * 128:(s + 1) * 128],
                                 dbc_bf[0:40, c0:c0 + cw], start=True, stop=True)
            nc.scalar.copy(Bb[:, si, :], pb[:])
            pc = bcp.tile([128, L], F32, tag="bc", name="pc")
            for c0, cw in CHS:
                nc.tensor.matmul(pc[:, c0:c0 + cw],
                                 sel[:, (16 + s) * 128:(17 + s) * 128],
                                 dbc_bf[0:40, c0:c0 + cw], start=True, stop=True)
            nc.scalar.copy(Cb[:, si, :], pc[:])
        # dBx is zero on every pad column (smask), and col L-1 is always a
        # pad column, so zeroing dA there makes the fwd carry self-killing
        # (h(seg end) = 0) and kills the bwd carry directly -- one dA tensor
        # serves both scan directions.
        nc.vector.memset(dA_f[:, :, L - 1:L], 0.0)
        nc.vector.tensor_mul(dBx[:], rep_ap(u[:], SGRP), Bb[:])
        nc.vector.tensor_tensor_scan(fl(hf[:]), fl(dA_f[:]), fl(dBx[:]),
                                     0.0, ALU.mult, ALU.add)
        nc.vector.tensor_tensor_scan(rev_ap(fl(hb[:])), rev_ap(fl(dA_f[:])),
                                     rev_ap(fl(dBx[:])), 0.0, ALU.mult, ALU.add)
        nc.vector.tensor_add(hf[:], hf[:], hb[:])
        nc.vector.tensor_mul(hf[:], hf[:], Cb[:])
        if g == 0:
            nc.vector.tensor_add(y[:], hf[:, 0, :], hf[:, 1, :])
        else:
            nc.vector.tensor_add(hf[:, 0, :], hf[:, 0, :], hf[:, 1, :])
            nc.vector.tensor_add(y[:], y[:], hf[:, 0, :])
    nc.vector.scalar_tensor_tensor(y[:], xic[:], WT[f"dp2{k}"][:], y[:],
                                   ALU.mult, ALU.add)
    ybf = ap_.tile([128, L], BF16, tag="ybf")
    nc.vector.tensor_mul(ybf[:], y[:], sz[:])
    for c0, cw in CHS:
        sl = slice(c0, c0 + cw)
        p5 = pp.tile([128, 512], F32, tag="mm")
        nc.tensor.matmul(p5[:, :cw], WT[f"wout{k}"][:], ybf[:, sl],
                         start=True, stop=True)
        nc.vector.scalar_tensor_tensor(seq[:, sl], p5[:, :cw], WT[f"wob{k}"][:],
                                       seq[:, sl], ALU.add, ALU.add)


def _build_l2():
    L = LSEQ
    nc = _mk_nc()
    decls = _l2_weight_decls()
    dp = {n: nc.declare_dram_parameter(n, s, d, isOutput=False)
          for n, s, d in decls}
    eo_out = nc.declare_dram_parameter("eo", [128, 4], F32, isOutput=True)

    with tile.TileContext(nc) as tc:
        with tc.tile_pool(name="w", bufs=1) as wp, \
             tc.tile_pool(name="act", bufs=1) as ap_, \
             tc.tile_pool(name="grp", bufs=1) as gp, \
             tc.tile_pool(name="sm", bufs=1) as sm, \
             tc.tile_pool(name="ps", bufs=3, space="PSUM") as pp, \
             tc.tile_pool(name="bc", bufs=1, space="PSUM") as bcp:
            WT = {}
            for n, s, d in decls:
                tag = n
                WT[n] = wp.tile(list(s), d, tag=tag, name="t_" + n)
                nc.sync.dma_start(out=WT[n][:], in_=dp[n][:])
            ones = wp.tile([128, 1], F32, tag="ones_")
            nc.vector.memset(ones[:], 1.0 / 128.0)
            eps_t = wp.tile([1, 1], F32, tag="eps_")
            nc.vector.memset(eps_t[:], 1e-5)
            WT["eps_"] = eps_t
            onesf = wp.tile([1, 128], F32, tag="onesf_")
            nc.vector.memset(onesf[:], 1.0)
            WT["onesf_"] = onesf
            sel = WT["sel"]

            seq = wp.tile([128, L], F32, tag="seq")
            for c0, cw in CHS:
                sl = slice(c0, c0 + cw)
                pst = pp.tile([128, 512], F32, tag="mm")
                nc.tensor.matmul(pst[:, :cw], WT["win_a"][:], WT["xa"][:, sl],
                                 start=True, stop=False)
                nc.tensor.matmul(pst[:, :cw], WT["win_b"][:], WT["xb"][:, sl],
                                 start=False, stop=True)
                nc.scalar.copy(seq[:, sl], pst[:, :cw])

            for k in range(4):
                _mamba_block(nc, ap_, gp, sm, pp, bcp, WT, seq, ones, sel, k)

            xh = ap_.tile([128, L], F32, tag="xh")
            _layernorm(nc, ap_, sm, pp, seq, xh, WT["norm_g"], WT["norm_b"], ones, WT["eps_"], WT["onesf_"])
            nmt = ap_.tile([128, L], F32, tag="cacc", name="nmt")
            nc.vector.tensor_scalar(out=nmt[:], in0=WT["mask"][:], scalar1=1e9,
                                    scalar2=-1e9, op0=ALU.mult, op1=ALU.add)
            xm = ap_.tile([128, L], F32, tag="y")
            nc.vector.tensor_add(xm[:], xh[:], nmt[:])
            pmax = sm.tile([128, 1], F32, tag="pmax")
            nc.vector.tensor_reduce(pmax[:], xm[:], axis=AX.X, op=ALU.max)
            xs = ap_.tile([128, L], F32, tag="cacc")
            nc.vector.tensor_mul(xs[:], xh[:], WT["mask"][:])
            psm = sm.tile([128, 1], F32, tag="psm")
            nc.vector.tensor_reduce(psm[:], xs[:], axis=AX.X, op=ALU.add)
            pmean = sm.tile([128, 1], F32, tag="pmean")
            nc.vector.tensor_mul(pmean[:], psm[:], WT["invn"][:])
            pool_bf = sm.tile([128, 2], BF16, tag="pool_bf")
            nc.vector.tensor_copy(pool_bf[:, 0:1], pmax[:])
            nc.vector.tensor_copy(pool_bf[:, 1:2], pmean[:])
            ps_m = pp.tile([128, 4], F32, tag="mm")
            for m in range(4):
                nc.tensor.matmul(ps_m[:, m:m + 1],
                                 WT["pw1a"][:, m * 128:(m + 1) * 128],
                                 pool_bf[:, 0:1], start=True, stop=False)
                nc.tensor.matmul(ps_m[:, m:m + 1],
                                 WT["pw1b"][:, m * 128:(m + 1) * 128],
                                 pool_bf[:, 1:2], start=False, stop=True)
            h1 = sm.tile([128, 4], BF16, tag="h1")
            nc.scalar.activation(h1[:], ps_m[:], AF.Gelu)
            ps_o = pp.tile([128, 4], F32, tag="mm")
            for m in range(4):
                for kc in range(4):
                    nc.tensor.matmul(
                        ps_o[:, m:m + 1],
                        WT["pw2"][:, kc * 512 + m * 128:kc * 512 + (m + 1) * 128],
                        h1[:, kc:kc + 1], start=(kc == 0), stop=(kc == 3))
            eo_t = sm.tile([128, 4], F32, tag="eo_t")
            nc.scalar.copy(eo_t[:], ps_o[:])
            nc.sync.dma_start(out=eo_out[:], in_=eo_t[:])
    nc.compile()
    return nc


# ================================================================ LAUNCH 3
def _build_l3():
    nc = _mk_nc()
    din = {}
    def D(name, shape, dt=BF16):
        din[name] = nc.declare_dram_parameter(name, shape, dt, isOutput=False)
    D("gfT", [128, 24])
    D("gw1", [128, 3 * 12 * 512])
    D("gw2", [128, 3 * 4 * 512])
    D("dwl", [128, 4 * CPC_L]); D("dwm", [128, 4 * CPC_M]); D("dwh", [128, 4 * CPC_H])
    outs = {"ol": nc.declare_dram_parameter("ol", [CPC_L, 2], F32, isOutput=True),
            "om": nc.declare_dram_parameter("om", [CPC_M, 2], F32, isOutput=True),
            "oh": nc.declare_dram_parameter("oh", [CPC_H, 2], F32, isOutput=True)}

    with tile.TileContext(nc) as tc:
        with tc.tile_pool(name="w", bufs=1) as wp, \
             tc.tile_pool(name="sm", bufs=1) as sm, \
             tc.tile_pool(name="ps", bufs=2, space="PSUM") as pp:
            WT = {}
            for name, t in din.items():
                WT[name] = wp.tile(list(t.shape), t.dtype, tag=name, name="t_" + name)
                nc.sync.dma_start(out=WT[name][:], in_=t[:])
            f2s = []
            for i in range(3):
                psf = pp.tile([128, 4, 2], F32, tag="mm")
                for m in range(4):
                    for kc in range(12):
                        w = WT["gw1"][:, ((i * 12 + kc) * 4 + m) * 128:
                                      ((i * 12 + kc) * 4 + m + 1) * 128]
                        nc.tensor.matmul(psf[:, m, :], w,
                                         WT["gfT"][:, kc * 2:(kc + 1) * 2],
                                         start=(kc == 0), stop=(kc == 11))
                f_bf = wp.tile([128, 4, 2], BF16, tag=f"f_bf{i}")
                nc.scalar.activation(f_bf[:], psf[:], AF.Gelu)
                ps2 = pp.tile([128, 4, 2], F32, tag="mm")
                for m in range(4):
                    for kc in range(4):
                        w = WT["gw2"][:, ((i * 4 + kc) * 4 + m) * 128:
                                      ((i * 4 + kc) * 4 + m + 1) * 128]
                        nc.tensor.matmul(ps2[:, m, :], w, f_bf[:, kc, :],
                                         start=(kc == 0), stop=(kc == 3))
                f2 = wp.tile([128, 4, 2], BF16, tag=f"f2_{i}")
                nc.vector.tensor_copy(f2[:], ps2[:])
                f2s.append(f2)
            # dec: low <- f3 (i=2), mid <- f2 (i=1), high <- f1 (i=0)
            for nm, cpc, fi in (("l", CPC_L, 2), ("m", CPC_M, 1), ("h", CPC_H, 0)):
                mcs = [(m0, min(128, cpc - m0)) for m0 in range(0, cpc, 128)]
                for m0, mw in mcs:
                    psd = pp.tile([128, 2], F32, tag="mm")
                    for kc in range(4):
                        w = WT["dw" + nm][:, kc * cpc + m0: kc * cpc + m0 + mw]
                        nc.tensor.matmul(psd[:mw, :], w, f2s[fi][:, kc, :],
                                         start=(kc == 0), stop=(kc == 3))
                    ot = sm.tile([128, 2], F32, tag="ot")
                    nc.scalar.copy(ot[:mw, :], psd[:mw, :])
                    nc.sync.dma_start(out=outs["o" + nm][m0:m0 + mw, :],
                                      in_=ot[:mw, :])
    nc.compile()
    return nc


# ================================================================ host packing
def _prep_unit(sampled_b, enhT_b, order_b, win):
    """Build xa/xb/mask/nmask/invn for one (b,res) unit."""
    n = sampled_b.shape[0]
    xin = np.zeros((134, LSEQ), np.float32)
    xin[0:6, 1:n + 1] = sampled_b[order_b].T
    xin[6:134, 1:n + 1] = enhT_b
    # oip_e column: v solving win.T @ v = oip_e
    return xin


def _pack_l2_core(xin, v_e, n, enc_p):
    L = LSEQ
    xin = xin.copy()
    xin[:, n + 1] = v_e
    mask = np.zeros((128, L), np.float32); mask[:, 1:n + 1] = 1.0
    smask = np.zeros((128, L), np.float32); smask[:, 0:n + 2] = 1.0

    m = {"xa": _bf(xin[0:128]), "xb": _bf(xin[128:134]),
         "mask": _bf(mask), "smask": _bf(smask),
         "invn": _f32(np.full((128, 1), 1.0 / n))}
    w_in = _f32(enc_p['in']['w'])
    m["win_a"] = _bf(w_in[0:128]); m["win_b"] = _bf(w_in[128:134])
    for k, blk in enumerate(enc_p['blocks']):
        m[f"ln_g{k}"] = _col(blk['ln_g']); m[f"ln_b{k}"] = _col(blk['ln_b'])
        m[f"wim{k}"] = _bf(blk['in']['w'])
        m[f"cw{k}"] = _f32(blk['conv_w']); m[f"cb{k}"] = _col(blk['conv_b'])
        m[f"xpw{k}"] = _bf(blk['xp_w'])
        m[f"dtw{k}"] = _bf(blk['dt']['w'])       # [R=8, di=128] is lhsT already
        m[f"dtbn{k}"] = _col(-np.asarray(blk['dt']['b'], np.float32))
        m[f"dp2{k}"] = _col(2.0 * np.asarray(blk['Dp'], np.float32))
        m[f"wout{k}"] = _bf(blk['out']['w']); m[f"wob{k}"] = _col(blk['out']['b'])
    m["norm_g"] = _col(enc_p['norm_g']); m["norm_b"] = _col(enc_p['norm_b'])
    seln = np.zeros((40, 32 * 128), np.float32)
    for idx in range(32):
        seln[8 + idx, idx * 128:(idx + 1) * 128] = 1.0
    m["sel"] = _bf(seln)
    pw1 = _f32(enc_p['out']['l1']['w'])
    m["pw1a"] = _bf(pw1[0:128]); m["pw1b"] = _bf(pw1[128:256])
    pw2 = _f32(enc_p['out']['l2']['w'])
    m["pw2"] = _bf(np.concatenate([pw2[kc * 128:(kc + 1) * 128, :]
                                   for kc in range(4)], axis=1))
    return m


def _dummy_l2_core():
    z = {n: np.zeros(s, BF if d == BF16 else np.float32)
         for n, s, d in _l2_weight_decls()}
    z["invn"][:] = 1.0
    return z


def kernel(arch_points, arch_blocks, params):
    params = _np_tree(params)
    arch_blocks = np.asarray(arch_blocks, np.float32)

    res_cfg = [("low", LOW), ("mid", MID), ("high", HIGH)]
    units = []           # list of dicts in order (res, b)
    pair_feats = []      # [7, n*32] per unit
    for rname, total in res_cfg:
        per = max(1, total // T)
        sampled = _fps(arch_blocks, per)            # [B, N, 6]
        xyz, nrm = sampled[..., :3], sampled[..., 3:6]
        knn = _knn(xyz)
        order = _zorder(xyz)
        for b in range(B):
            o = order[b]
            nb = knn[b][o]                           # [N, 32] (z-ordered rows)
            rel = xyz[b][nb] - xyz[b][o][:, None, :]
            nnb = nrm[b][nb]
            var = np.abs(1.0 - np.sum(nrm[b][o][:, None, :] * nnb, -1))
            pf = np.concatenate([rel.reshape(-1, 3),
                                 nnb.reshape(-1, 3),
                                 var.reshape(-1, 1)], axis=1).T  # [7, N*32]
            pair_feats.append(pf.astype(np.float32))
            units.append({"res": rname, "b": b, "n": sampled.shape[1],
                          "sampled": sampled[b], "order": o})

    # ---- launch 1
    allpairs = np.concatenate(pair_feats, axis=1)    # [7, NPTS*32]
    run1 = _runner("l1", _build_l1)
    enh_p = params['enh']
    w1 = {"w1c": _bf(enh_p['coord']['l1']['w']),
          "w1n": _bf(enh_p['normal']['l1']['w']),
          "w2c": _bf(enh_p['coord']['l2']['w']),
          "w2n": _bf(enh_p['normal']['l2']['w']),
          "w3a": _bf(enh_p['out']['l1']['w'][0:128]),
          "w3b": _bf(enh_p['out']['l1']['w'][128:256]),
          "w4": _bf(enh_p['out']['l2']['w'])}
    in_maps1 = []
    for c in range(NCORES):
        sl = slice(c * PAIRS_PER_CORE, (c + 1) * PAIRS_PER_CORE)
        pr = np.zeros((36, PAIRS_PER_CORE), np.float32)
        pr[0:3] = allpairs[0:3, sl]
        pr[32:36] = allpairs[3:7, sl]
        in_maps1.append({"pr": _bf(pr), **w1})
    res1 = run1(in_maps1)
    enhT = np.concatenate([res1[c]["enh"] for c in range(NCORES)], axis=1)  # [128, NPTS]

    # ---- launch 2
    run2 = _runner("l2", _build_l2)
    in_maps2 = []
    off = 0
    enc_key = {"low": "enc_low", "mid": "enc_mid", "high": "enc_high"}
    for u in units:
        n = u["n"]
        enc_p = params[enc_key[u["res"]]]
        xin = _prep_unit(u["sampled"], enhT[:, off:off + n], u["order"],
                         enc_p['in']['w'])
        off += n
        oip_e = (np.asarray(enc_p['oip']['w'], np.float32)[0]
                 + np.asarray(enc_p['oip']['b'], np.float32))
        w_in = _f32(enc_p['in']['w'])
        v_e = np.linalg.lstsq(w_in.T, oip_e - _f32(enc_p['in']['b']),
                              rcond=None)[0]
        in_maps2.append(_pack_l2_core(xin, v_e, n, enc_p))
    while len(in_maps2) < NCORES:
        in_maps2.append(_dummy_l2_core())
    res2 = run2(in_maps2)
    eo = [res2[c]["eo"].T.flatten() for c in range(6)]   # [512] each
    # units order: low_b0, low_b1, mid_b0, mid_b1, high_b0, high_b1
    gf = np.stack([np.concatenate([eo[0], eo[2], eo[4]]),
                   np.concatenate([eo[1], eo[3], eo[5]])], axis=0)  # [2,1536]

    # ---- launch 3
    run3 = _runner("l3", _build_l3)
    gfT = np.zeros((128, 24), np.float32)
    for kc in range(12):
        gfT[:, kc * 2:(kc + 1) * 2] = gf[:, kc * 128:(kc + 1) * 128].T
    gw1 = np.zeros((128, 3 * 12 * 512), np.float32)
    for i, gk in enumerate(("g1", "g2", "g3")):
        W1 = _f32(params[gk]['l1']['w'])
        for kc in range(12):
            for mch in range(4):
                gw1[:, ((i * 12 + kc) * 4 + mch) * 128:((i * 12 + kc) * 4 + mch + 1) * 128] = \
                    W1[kc * 128:(kc + 1) * 128, mch * 128:(mch + 1) * 128]
    gw2 = np.zeros((128, 3 * 4 * 512), np.float32)
    for i, gk in enumerate(("g1", "g2", "g3")):
        W2 = _f32(params[gk]['l2']['w'])
        for kc in range(4):
            for mch in range(4):
                gw2[:, ((i * 4 + kc) * 4 + mch) * 128:((i * 4 + kc) * 4 + mch + 1) * 128] = \
                    W2[kc * 128:(kc + 1) * 128, mch * 128:(mch + 1) * 128]
    in_maps3 = []
    dec = params['dec']
    for c in range(NCORES):
        m = {"gfT": _bf(gfT), "gw1": _bf(gw1), "gw2": _bf(gw2)}
        for nm, key, cpc in (("dwl", "low", CPC_L), ("dwm", "mid", CPC_M),
                             ("dwh", "high", CPC_H)):
            W = _f32(dec[key]['w'])                  # [512, LOWx3]
            sl = W[:, c * cpc:(c + 1) * cpc]
            m[nm] = _bf(np.concatenate([sl[kc * 128:(kc + 1) * 128, :]
                                        for kc in range(4)], axis=1))
        in_maps3.append(m)
    res3 = run3(in_maps3)
    dlo = np.concatenate([res3[c]["ol"].T for c in range(NCORES)], axis=1)
    dmd = np.concatenate([res3[c]["om"].T for c in range(NCORES)], axis=1)
    dhi = np.concatenate([res3[c]["oh"].T for c in range(NCORES)], axis=1)

    pl = dlo.reshape(B, LOW, 3)
    pm = _expand(pl, MID) + dmd.reshape(B, MID, 3)
    ph = _expand(pm, HIGH) + dhi.reshape(B, HIGH, 3)
    return (pl.astype(np.float32), pm.astype(np.float32),
            ph.astype(np.float32), gf.astype(np.float32))


# revision 26
# speedup vs baseline: 1.5076x; 1.0375x over previous
"""CMDenNet Trainium2 kernel: host prep (FPS/kNN/z-order/gathers) + 3 SPMD Bass launches.

L1: enhancer MLPs, 8-way row split (896 points x 32 neighbors per core).
L2: encoders (4 bidirectional-Mamba blocks + pooling + out MLP); one unit per
    core, zero-padded to a uniform L=2052 so a single static program serves all
    cores (6 real units + 2 dummy cores).
L3: heads (g1/g2/g3 MLPs replicated, dec projections column-split 8 ways).

All data-dependent indexing (FPS, kNN, Morton order) is computed on host and
folded into the input packing; device kernels are shape-static, so programs
compile once per process and are reused across calls.
"""
import sys
import numpy as np

sys.path.insert(0, '/opt/trn_rl_repo')

import ml_dtypes
import concourse.bass as bass
import concourse.tile as tile
from concourse import bacc, mybir
from concourse import bass2jax

F32 = mybir.dt.float32
BF16 = mybir.dt.bfloat16
AF = mybir.ActivationFunctionType
ALU = mybir.AluOpType
AX = mybir.AxisListType
BF = ml_dtypes.bfloat16

H = 128; G = 512; K_NN = 32; RES = 64; S = 16; R = 8; KCONV = 4
LOW, MID, HIGH = 512, 1024, 2048
B, T, P, PD = 2, 16, 256, 6
NCORES = 8
LSEQ = 2052
NPTS = (LOW + MID + HIGH) * B          # 7168
PTS_PER_CORE = NPTS // NCORES          # 896
PAIRS_PER_CORE = PTS_PER_CORE * K_NN   # 28672
SGRP = 2                               # s-states per SSM group
CHS = [(0, 512), (512, 512), (1024, 512), (1536, 512), (2048, 4)]
CPC_L, CPC_M, CPC_H = LOW * 3 // NCORES, MID * 3 // NCORES, HIGH * 3 // NCORES


# ---------------------------------------------------------------- host math
def _fps(blocks, n):
    b, t, pp, d = blocks.shape
    pts = blocks.reshape(b * t, pp, d)
    xyz = pts[..., :3]
    mind = np.sum((xyz - xyz[:, :1]) ** 2, -1)
    idxs = [np.zeros((b * t,), np.int32)]
    for _ in range(n - 1):
        nxt = np.argmax(mind, axis=1)
        sel = np.take_along_axis(xyz, nxt[:, None, None], axis=1)
        mind = np.minimum(mind, np.sum((xyz - sel) ** 2, -1))
        idxs.append(nxt.astype(np.int32))
    idx = np.stack(idxs, 0).T
    out = np.take_along_axis(pts, idx[..., None], axis=1)
    return out.reshape(b, t * n, d)


def _knn(xyz):
    bsz, n, _ = xyz.shape
    sq = np.sum(xyz * xyz, -1)
    out = np.empty((bsz, n, K_NN), np.int64)
    for bi in range(bsz):
        d2 = sq[bi][:, None] + sq[bi][None, :] - 2.0 * (xyz[bi] @ xyz[bi].T)
        idx = np.argsort(d2, axis=1, kind='stable')[:, :K_NN + 1]
        out[bi] = idx[:, 1:]
    return out


def _zorder(xyz):
    mn = xyz.min(axis=1, keepdims=True)
    mx = xyz.max(axis=1, keepdims=True)
    g = np.clip(((xyz - mn) / (mx - mn + 1e-9) * RES).astype(np.int32), 0, RES - 1)
    code = np.zeros(g.shape[:2], np.int32)
    for bit in range(6):
        for ax in range(3):
            code = code | (((g[..., ax] >> bit) & 1) << (3 * bit + ax))
    return np.argsort(code, axis=1, kind='stable')


def _expand(pts, out_n):
    rep = -(-out_n // pts.shape[1])
    return np.repeat(pts, rep, axis=1)[:, :out_n]


def _np_tree(tree):
    if isinstance(tree, dict):
        return {k: _np_tree(v) for k, v in tree.items()}
    if isinstance(tree, (list, tuple)):
        return [_np_tree(v) for v in tree]
    return np.asarray(tree)


def _bf(x):
    return np.ascontiguousarray(np.asarray(x, np.float32).astype(BF))


def _f32(x):
    return np.ascontiguousarray(np.asarray(x, np.float32))


def _col(x):
    return _f32(np.asarray(x).reshape(128, 1))


# ---------------------------------------------------------------- bass helpers
def _mk_nc():
    return bacc.Bacc("TRN2", target_bir_lowering=False, debug=False,
                     num_devices=NCORES)


def rev_ap(ap_in):
    (pstep, pcnt), (estep, ecnt) = ap_in.ap
    return bass.AP(tensor=ap_in.tensor, offset=ap_in.offset + (ecnt - 1) * estep,
                   ap=[[pstep, pcnt], [-estep, ecnt]])


def rep_ap(ap_in, n):
    """[P, L] viewed as [P, n, L] with the middle axis stride 0."""
    (pstep, pcnt), (estep, ecnt) = ap_in.ap
    return bass.AP(tensor=ap_in.tensor, offset=ap_in.offset,
                   ap=[[pstep, pcnt], [0, n], [estep, ecnt]])


_RUNNERS = {}


def _runner(key, build_fn):
    if key in _RUNNERS:
        return _RUNNERS[key]
    import jax
    from jax.sharding import Mesh, PartitionSpec
    from jax.experimental.shard_map import shard_map

    nc = build_fn()
    bass2jax.install_neuronx_cc_hook()
    partition_name = nc.partition_id_tensor.name if nc.partition_id_tensor else None
    in_names, out_names, out_avals, zero_shapes = [], [], [], []
    for alloc in nc.m.functions[0].allocations:
        if not isinstance(alloc, mybir.MemoryLocationSet):
            continue
        name = alloc.memorylocations[0].name
        if alloc.kind == "ExternalInput":
            if name != partition_name:
                in_names.append(name)
        elif alloc.kind == "ExternalOutput":
            out_names.append(name)
            shape = tuple(alloc.tensor_shape)
            dtype = mybir.dt.np(alloc.dtype)
            out_avals.append(jax.core.ShapedArray(shape, dtype))
            zero_shapes.append((shape, dtype))
    n_params = len(in_names)
    n_outs = len(out_avals)
    in_names_all = list(in_names) + list(out_names)
    if partition_name is not None:
        in_names_all.append(partition_name)

    def _body(*args):
        operands = list(args)
        if partition_name is not None:
            operands.append(bass2jax.partition_id_tensor())
        outs = bass2jax._bass_exec_p.bind(
            *operands, out_avals=tuple(out_avals), in_names=tuple(in_names_all),
            out_names=tuple(out_names), lowering_input_output_aliases=(),
            sim_require_finite=False, sim_require_nnan=False, nc=nc)
        return tuple(outs)

    donate = tuple(range(n_params, n_params + n_outs))
    devices = jax.devices()[:NCORES]
    mesh = Mesh(np.asarray(devices), ("core",))
    jfn = jax.jit(shard_map(_body, mesh=mesh,
                            in_specs=(PartitionSpec("core"),) * (n_params + n_outs),
                            out_specs=(PartitionSpec("core"),) * n_outs,
                            check_rep=False),
                  donate_argnums=donate, keep_unused=True)

    def run(in_maps):
        per_core = [[np.ascontiguousarray(m[n]) for n in in_names] for m in in_maps]
        concat_in = [np.concatenate([per_core[c][i] for c in range(NCORES)], axis=0)
                     for i in range(n_params)]
        concat_zero = [np.zeros((NCORES * s[0], *s[1:]), d) for s, d in zero_shapes]
        outs = jfn(*concat_in, *concat_zero)
        outs = [np.asarray(o) for o in outs]
        return [{name: outs[i].reshape(NCORES, *out_avals[i].shape)[c]
                 for i, name in enumerate(out_names)} for c in range(NCORES)]

    _RUNNERS[key] = run
    return run


# ================================================================ LAUNCH 1
def _build_l1():
    NPAIR, NPT, CH = PAIRS_PER_CORE, PTS_PER_CORE, 512
    nc = _mk_nc()
    pr_in = nc.declare_dram_parameter("pr", [36, NPAIR], BF16, isOutput=False)
    names = [("w1c", [3, 128]), ("w1n", [4, 128]), ("w2c", [128, 128]),
             ("w2n", [128, 128]), ("w3a", [128, 128]), ("w3b", [128, 128]),
             ("w4", [128, 128])]
    dins = {n: nc.declare_dram_parameter(n, s, BF16, isOutput=False) for n, s in names}
    enh_out = nc.declare_dram_parameter("enh", [128, NPT], F32, isOutput=True)

    with tile.TileContext(nc) as tc:
        with tc.tile_pool(name="w", bufs=1) as wp, \
             tc.tile_pool(name="mid", bufs=4) as mp, \
             tc.tile_pool(name="ps", bufs=6, space="PSUM") as pp:
            WT = {}
            for n, s in names:
                if n == "w1n":
                    w1n_t = wp.tile([36, 128], BF16, tag=n, name="t_" + n)
                    nc.sync.dma_start(out=w1n_t[32:36, :], in_=dins[n][:])
                    WT[n] = w1n_t
                else:
                    WT[n] = wp.tile(s, BF16, tag=n, name="t_" + n)
                    nc.sync.dma_start(out=WT[n][:], in_=dins[n][:])
            pr = wp.tile([36, NPAIR], BF16, tag="pr")
            nc.sync.dma_start(out=pr[:], in_=pr_in[:])
            cf = wp.tile([128, NPT], BF16, tag="cf")
            nf = wp.tile([128, NPT], BF16, tag="nf")

            for j0 in range(0, NPAIR, CH):
                npair = min(CH, NPAIR - j0)
                npt = npair // K_NN
                p0 = j0 // K_NN
                # interleave the two branch chains so PE works on one branch
                # while ACT gelus the other (avoids PE<->ACT ping-pong)
                ps1c = pp.tile([128, CH], F32, tag="mm", name="ps1c")
                nc.tensor.matmul(ps1c[:, :npair], WT["w1c"][:],
                                 pr[0:3, j0:j0 + npair], start=True, stop=True)
                ps1n = pp.tile([128, CH], F32, tag="mm", name="ps1n")
                nc.tensor.matmul(ps1n[:, :npair], WT["w1n"][32:36, :],
                                 pr[32:36, j0:j0 + npair], start=True, stop=True)
                a1c = mp.tile([128, CH], BF16, tag="a1", name="a1c")
                nc.scalar.activation(a1c[:, :npair], ps1c[:, :npair], AF.Gelu)
                a1n = mp.tile([128, CH], BF16, tag="a1", name="a1n")
                nc.scalar.activation(a1n[:, :npair], ps1n[:, :npair], AF.Gelu)
                ps2c = pp.tile([128, CH], F32, tag="mm", name="ps2c")
                nc.tensor.matmul(ps2c[:, :npair], WT["w2c"][:], a1c[:, :npair],
                                 start=True, stop=True)
                ps2n = pp.tile([128, CH], F32, tag="mm", name="ps2n")
                nc.tensor.matmul(ps2n[:, :npair], WT["w2n"][:], a1n[:, :npair],
                                 start=True, stop=True)
                nc.vector.tensor_reduce(
                    cf[:, p0:p0 + npt],
                    ps2c[:, :npair].rearrange("p (n k) -> p n k", k=K_NN),
                    axis=AX.X, op=ALU.max)
                nc.vector.tensor_reduce(
                    nf[:, p0:p0 + npt],
                    ps2n[:, :npair].rearrange("p (n k) -> p n k", k=K_NN),
                    axis=AX.X, op=ALU.max)
            for j0 in range(0, NPT, CH):
                npt = min(CH, NPT - j0)
                ps3 = pp.tile([128, CH], F32, tag="mm")
                nc.tensor.matmul(ps3[:, :npt], WT["w3a"][:], cf[:, j0:j0 + npt],
                                 start=True, stop=False)
                nc.tensor.matmul(ps3[:, :npt], WT["w3b"][:], nf[:, j0:j0 + npt],
                                 start=False, stop=True)
                a3 = mp.tile([128, CH], BF16, tag="a3")
                nc.scalar.activation(a3[:, :npt], ps3[:, :npt], AF.Gelu)
                ps4 = pp.tile([128, CH], F32, tag="mm")
                nc.tensor.matmul(ps4[:, :npt], WT["w4"][:], a3[:, :npt],
                                 start=True, stop=True)
                o = mp.tile([128, CH], F32, tag="o")
                nc.scalar.copy(o[:, :npt], ps4[:, :npt])
                nc.sync.dma_start(out=enh_out[:, j0:j0 + npt], in_=o[:, :npt])
    nc.compile()
    return nc


# ================================================================ LAUNCH 2
_L2_WNAMES = []


def _l2_weight_decls():
    names = [("xa", [128, LSEQ], BF16), ("xb", [6, LSEQ], BF16),
             ("mask", [128, LSEQ], BF16),
             ("smask", [128, LSEQ], BF16),
             ("invn", [128, 1], F32),
             ("win_a", [128, 128], BF16), ("win_b", [6, 128], BF16)]
    for k in range(4):
        names += [(f"ln_g{k}", [128, 1], F32), (f"ln_b{k}", [128, 1], F32),
                  (f"wim{k}", [128, 256], BF16), (f"cw{k}", [128, 4], F32),
                  (f"cb{k}", [128, 1], F32), (f"xpw{k}", [128, 40], BF16),
                  (f"dtw{k}", [8, 128], BF16), (f"dtbn{k}", [128, 1], F32),
                  (f"dp2{k}", [128, 1], F32), (f"wout{k}", [128, 128], BF16),
                  (f"wob{k}", [128, 1], F32)]
    names += [("norm_g", [128, 1], F32), ("norm_b", [128, 1], F32),
              ("sel", [40, 32 * 128], BF16),
              ("pw1a", [128, 512], BF16), ("pw1b", [128, 512], BF16),
              ("pw2", [128, 2048], BF16)]
    return names


def _layernorm(nc, ap_, sm, pp, seq, xh, g, b, ones, eps, onesf):
    L = LSEQ
    mu = sm.tile([1, L], F32, tag="ln_mu")
    mu2 = sm.tile([1, L], F32, tag="ln_mu2")
    std = sm.tile([1, L], F32, tag="ln_std")
    sq = ap_.tile([128, L], F32, tag="cacc")          # alias cacc slot
    nc.scalar.activation(sq[:], seq[:], AF.Square)
    for c0, cw in CHS:
        sl = slice(c0, c0 + cw)
        pm = pp.tile([1, 512], F32, tag="mm", name="pm")
        nc.tensor.matmul(pm[:, :cw], ones[:], seq[:, sl], start=True, stop=True)
        nc.vector.tensor_copy(mu[:, sl], pm[:, :cw])
        pm2 = pp.tile([1, 512], F32, tag="mm", name="pm2")
        nc.tensor.matmul(pm2[:, :cw], ones[:], sq[:, sl], start=True, stop=True)
        nc.vector.tensor_copy(mu2[:, sl], pm2[:, :cw])
    nc.vector.tensor_mul(std[:], mu[:], mu[:])
    nc.vector.tensor_sub(mu2[:], mu2[:], std[:])
    nc.scalar.activation(std[:], mu2[:], AF.Sqrt, bias=eps[:])
    rstd = mu2
    nc.vector.reciprocal(rstd[:], std[:])
    for c0, cw in CHS:
        sl = slice(c0, c0 + cw)
        mub = pp.tile([128, 512], F32, tag="mm", name="mub")
        nc.tensor.matmul(mub[:, :cw], onesf[:], mu[:, sl], start=True, stop=True)
        nc.vector.tensor_sub(xh[:, sl], seq[:, sl], mub[:, :cw])
        rstdb = pp.tile([128, 512], F32, tag="mm", name="rstdb")
        nc.tensor.matmul(rstdb[:, :cw], onesf[:], rstd[:, sl], start=True, stop=True)
        nc.vector.tensor_mul(xh[:, sl], xh[:, sl], rstdb[:, :cw])
    nc.vector.tensor_scalar(out=xh[:], in0=xh[:], scalar1=g[:], scalar2=b[:],
                            op0=ALU.mult, op1=ALU.add)


def _mamba_block(nc, ap_, gp, sm, pp, bcp, WT, seq, ones, sel, k):
    L = LSEQ
    xh = ap_.tile([128, L], F32, tag="xh")
    _layernorm(nc, ap_, sm, pp, seq, xh, WT[f"ln_g{k}"], WT[f"ln_b{k}"], ones, WT["eps_"], WT["onesf_"])
    xh_bf = ap_.tile([128, L], BF16, tag="ybf")       # alias ybf slot
    for c0, cw in CHS:
        nc.vector.tensor_copy(xh_bf[:, c0:c0 + cw], xh[:, c0:c0 + cw])
    xi = ap_.tile([128, L], BF16, tag="xi")
    sz = ap_.tile([128, L], BF16, tag="sz")
    for c0, cw in CHS:
        sl = slice(c0, c0 + cw)
        p1 = pp.tile([128, 512], F32, tag="mm")
        nc.tensor.matmul(p1[:, :cw], WT[f"wim{k}"][:, 0:128], xh_bf[:, sl],
                         start=True, stop=True)
        nc.scalar.copy(xi[:, sl], p1[:, :cw])
        p2 = pp.tile([128, 512], F32, tag="mm")
        nc.tensor.matmul(p2[:, :cw], WT[f"wim{k}"][:, 128:256], xh_bf[:, sl],
                         start=True, stop=True)
        nc.scalar.activation(sz[:, sl], p2[:, :cw], AF.Silu)
    cacc = ap_.tile([128, L], F32, tag="cacc")
    cw_t = WT[f"cw{k}"]
    nc.vector.tensor_scalar(out=cacc[:], in0=xi[:], scalar1=cw_t[:, 3:4],
                            scalar2=WT[f"cb{k}"][:], op0=ALU.mult, op1=ALU.add)
    for j in range(3):
        sh = 3 - j
        nc.vector.scalar_tensor_tensor(cacc[:, sh:], xi[:, :L - sh],
                                       cw_t[:, j:j + 1], cacc[:, sh:],
                                       ALU.mult, ALU.add)
    xic = ap_.tile([128, L], BF16, tag="xic")
    nc.scalar.activation(xic[:], cacc[:], AF.Silu)
    dbc_bf = ap_.tile([40, L], BF16, tag="dbc_bf")
    for c0, cw in CHS:
        sl = slice(c0, c0 + cw)
        p3 = pp.tile([40, 512], F32, tag="mm")
        nc.tensor.matmul(p3[:, :cw], WT[f"xpw{k}"][:], xic[:, sl],
                         start=True, stop=True)
        nc.scalar.copy(dbc_bf[:, sl], p3[:, :cw])
    # dt = softplus(p) via sigmoid/ln: dt_f holds ln(sigmoid(-p)) = -dt
    sig = ap_.tile([128, L], F32, tag="y", name="sig")
    dt_f = ap_.tile([128, L], F32, tag="dt_f")
    for c0, cw in CHS:
        sl = slice(c0, c0 + cw)
        p4 = pp.tile([128, 512], F32, tag="mm")
        nc.tensor.matmul(p4[:, :cw], WT[f"dtw{k}"][:], dbc_bf[0:8, sl],
                         start=True, stop=True)
        nc.scalar.activation(sig[:, sl], p4[:, :cw], AF.Sigmoid,
                             scale=-1.0, bias=WT[f"dtbn{k}"][:])
    nc.scalar.activation(dt_f[:], sig[:], AF.Ln)
    u = ap_.tile([128, L], BF16, tag="u")
    nc.vector.tensor_scalar_mul(u[:], dt_f[:], -1.0)
    nc.vector.tensor_mul(u[:], u[:], xic[:])
    nc.vector.tensor_mul(u[:], u[:], WT["smask"][:])
    y = ap_.tile([128, L], F32, tag="y")
    fl = lambda t: t.rearrange("p s l -> p (s l)")
    for g in range(S // SGRP):
        dA_f = gp.tile([128, SGRP, L], BF16, tag="dA_f", bufs=2)
        dBx = gp.tile([128, SGRP, L], BF16, tag="dBx", bufs=2)
        hf = gp.tile([128, SGRP, L], BF16, tag="hf")
        hb = gp.tile([128, SGRP, L], BF16, tag="dA_b", name="hb")
        Bb = gp.tile([128, SGRP, L], BF16, tag="Bb", bufs=2)
        Cb = gp.tile([128, SGRP, L], BF16, tag="Cb", bufs=2)
        for si in range(SGRP):
            s = g * SGRP + si
            nc.scalar.activation(dA_f[:, si, :], dt_f[:], AF.Exp,
                                 scale=float(s + 1))
            pb = bcp.tile([128, L], F32, tag="bc", name="pb")
            for c0, cw in CHS:
                nc.tensor.matmul(pb[:, c0:c0 + cw], sel[:, s * 128:(s + 1) * 128],
                                 dbc_bf[0:40, c0:c0 + cw], start=True, stop=True)
            nc.scalar.copy(Bb[:, si, :], pb[:])
            pc = bcp.tile([128, L], F32, tag="bc", name="pc")
            for c0, cw in CHS:
                nc.tensor.matmul(pc[:, c0:c0 + cw],
                                 sel[:, (16 + s) * 128:(17 + s) * 128],
                                 dbc_bf[0:40, c0:c0 + cw], start=True, stop=True)
            nc.scalar.copy(Cb[:, si, :], pc[:])
        # dBx is zero on every pad column (smask), and col L-1 is always a
        # pad column, so zeroing dA there makes the fwd carry self-killing
        # (h(seg end) = 0) and kills the bwd carry directly -- one dA tensor
        # serves both scan directions.
        nc.vector.memset(dA_f[:, :, L - 1:L], 0.0)
        nc.vector.tensor_mul(dBx[:], rep_ap(u[:], SGRP), Bb[:])
        nc.vector.tensor_tensor_scan(fl(hf[:]), fl(dA_f[:]), fl(dBx[:]),
                                     0.0, ALU.mult, ALU.add)
        nc.vector.tensor_tensor_scan(rev_ap(fl(hb[:])), rev_ap(fl(dA_f[:])),
                                     rev_ap(fl(dBx[:])), 0.0, ALU.mult, ALU.add)
        nc.vector.tensor_add(hf[:], hf[:], hb[:])
        nc.vector.tensor_mul(hf[:], hf[:], Cb[:])
        if g == 0:
            nc.vector.tensor_add(y[:], hf[:, 0, :], hf[:, 1, :])
        else:
            nc.vector.tensor_add(hf[:, 0, :], hf[:, 0, :], hf[:, 1, :])
            nc.vector.tensor_add(y[:], y[:], hf[:, 0, :])
    nc.vector.scalar_tensor_tensor(y[:], xic[:], WT[f"dp2{k}"][:], y[:],
                                   ALU.mult, ALU.add)
    ybf = ap_.tile([128, L], BF16, tag="ybf")
    nc.vector.tensor_mul(ybf[:], y[:], sz[:])
    for c0, cw in CHS:
        sl = slice(c0, c0 + cw)
        p5 = pp.tile([128, 512], F32, tag="mm")
        nc.tensor.matmul(p5[:, :cw], WT[f"wout{k}"][:], ybf[:, sl],
                         start=True, stop=True)
        nc.vector.scalar_tensor_tensor(seq[:, sl], p5[:, :cw], WT[f"wob{k}"][:],
                                       seq[:, sl], ALU.add, ALU.add)


def _build_l2():
    L = LSEQ
    nc = _mk_nc()
    decls = _l2_weight_decls()
    dp = {n: nc.declare_dram_parameter(n, s, d, isOutput=False)
          for n, s, d in decls}
    eo_out = nc.declare_dram_parameter("eo", [128, 4], F32, isOutput=True)

    with tile.TileContext(nc) as tc:
        with tc.tile_pool(name="w", bufs=1) as wp, \
             tc.tile_pool(name="act", bufs=1) as ap_, \
             tc.tile_pool(name="grp", bufs=1) as gp, \
             tc.tile_pool(name="sm", bufs=1) as sm, \
             tc.tile_pool(name="ps", bufs=3, space="PSUM") as pp, \
             tc.tile_pool(name="bc", bufs=1, space="PSUM") as bcp:
            WT = {}
            for n, s, d in decls:
                tag = n
                WT[n] = wp.tile(list(s), d, tag=tag, name="t_" + n)
                nc.sync.dma_start(out=WT[n][:], in_=dp[n][:])
            ones = wp.tile([128, 1], F32, tag="ones_")
            nc.vector.memset(ones[:], 1.0 / 128.0)
            eps_t = wp.tile([1, 1], F32, tag="eps_")
            nc.vector.memset(eps_t[:], 1e-5)
            WT["eps_"] = eps_t
            onesf = wp.tile([1, 128], F32, tag="onesf_")
            nc.vector.memset(onesf[:], 1.0)
            WT["onesf_"] = onesf
            sel = WT["sel"]

            seq = wp.tile([128, L], F32, tag="seq")
            for c0, cw in CHS:
                sl = slice(c0, c0 + cw)
                pst = pp.tile([128, 512], F32, tag="mm")
                nc.tensor.matmul(pst[:, :cw], WT["win_a"][:], WT["xa"][:, sl],
                                 start=True, stop=False)
                nc.tensor.matmul(pst[:, :cw], WT["win_b"][:], WT["xb"][:, sl],
                                 start=False, stop=True)
                nc.scalar.copy(seq[:, sl], pst[:, :cw])

            for k in range(4):
                _mamba_block(nc, ap_, gp, sm, pp, bcp, WT, seq, ones, sel, k)

            xh = ap_.tile([128, L], F32, tag="xh")
            _layernorm(nc, ap_, sm, pp, seq, xh, WT["norm_g"], WT["norm_b"], ones, WT["eps_"], WT["onesf_"])
            nmt = ap_.tile([128, L], F32, tag="cacc", name="nmt")
            nc.vector.tensor_scalar(out=nmt[:], in0=WT["mask"][:], scalar1=1e9,
                                    scalar2=-1e9, op0=ALU.mult, op1=ALU.add)
            xm = ap_.tile([128, L], F32, tag="y")
            nc.vector.tensor_add(xm[:], xh[:], nmt[:])
            pmax = sm.tile([128, 1], F32, tag="pmax")
            nc.vector.tensor_reduce(pmax[:], xm[:], axis=AX.X, op=ALU.max)
            xs = ap_.tile([128, L], F32, tag="cacc")
            nc.vector.tensor_mul(xs[:], xh[:], WT["mask"][:])
            psm = sm.tile([128, 1], F32, tag="psm")
            nc.vector.tensor_reduce(psm[:], xs[:], axis=AX.X, op=ALU.add)
            pmean = sm.tile([128, 1], F32, tag="pmean")
            nc.vector.tensor_mul(pmean[:], psm[:], WT["invn"][:])
            pool_bf = sm.tile([128, 2], BF16, tag="pool_bf")
            nc.vector.tensor_copy(pool_bf[:, 0:1], pmax[:])
            nc.vector.tensor_copy(pool_bf[:, 1:2], pmean[:])
            ps_m = pp.tile([128, 4], F32, tag="mm")
            for m in range(4):
                nc.tensor.matmul(ps_m[:, m:m + 1],
                                 WT["pw1a"][:, m * 128:(m + 1) * 128],
                                 pool_bf[:, 0:1], start=True, stop=False)
                nc.tensor.matmul(ps_m[:, m:m + 1],
                                 WT["pw1b"][:, m * 128:(m + 1) * 128],
                                 pool_bf[:, 1:2], start=False, stop=True)
            h1 = sm.tile([128, 4], BF16, tag="h1")
            nc.scalar.activation(h1[:], ps_m[:], AF.Gelu)
            ps_o = pp.tile([128, 4], F32, tag="mm")
            for m in range(4):
                for kc in range(4):
                    nc.tensor.matmul(
                        ps_o[:, m:m + 1],
                        WT["pw2"][:, kc * 512 + m * 128:kc * 512 + (m + 1) * 128],
                        h1[:, kc:kc + 1], start=(kc == 0), stop=(kc == 3))
            eo_t = sm.tile([128, 4], F32, tag="eo_t")
            nc.scalar.copy(eo_t[:], ps_o[:])
            nc.sync.dma_start(out=eo_out[:], in_=eo_t[:])
    nc.compile()
    return nc


# ================================================================ LAUNCH 3
def _build_l3():
    nc = _mk_nc()
    din = {}
    def D(name, shape, dt=BF16):
        din[name] = nc.declare_dram_parameter(name, shape, dt, isOutput=False)
    D("gfT", [128, 24])
    D("gw1", [128, 3 * 12 * 512])
    D("gw2", [128, 3 * 4 * 512])
    D("dwl", [128, 4 * CPC_L]); D("dwm", [128, 4 * CPC_M]); D("dwh", [128, 4 * CPC_H])
    outs = {"ol": nc.declare_dram_parameter("ol", [CPC_L, 2], F32, isOutput=True),
            "om": nc.declare_dram_parameter("om", [CPC_M, 2], F32, isOutput=True),
            "oh": nc.declare_dram_parameter("oh", [CPC_H, 2], F32, isOutput=True)}

    with tile.TileContext(nc) as tc:
        with tc.tile_pool(name="w", bufs=1) as wp, \
             tc.tile_pool(name="sm", bufs=1) as sm, \
             tc.tile_pool(name="ps", bufs=2, space="PSUM") as pp:
            WT = {}
            for name, t in din.items():
                WT[name] = wp.tile(list(t.shape), t.dtype, tag=name, name="t_" + name)
                nc.sync.dma_start(out=WT[name][:], in_=t[:])
            f2s = []
            for i in range(3):
                psf = pp.tile([128, 4, 2], F32, tag="mm")
                for m in range(4):
                    for kc in range(12):
                        w = WT["gw1"][:, ((i * 12 + kc) * 4 + m) * 128:
                                      ((i * 12 + kc) * 4 + m + 1) * 128]
                        nc.tensor.matmul(psf[:, m, :], w,
                                         WT["gfT"][:, kc * 2:(kc + 1) * 2],
                                         start=(kc == 0), stop=(kc == 11))
                f_bf = wp.tile([128, 4, 2], BF16, tag=f"f_bf{i}")
                nc.scalar.activation(f_bf[:], psf[:], AF.Gelu)
                ps2 = pp.tile([128, 4, 2], F32, tag="mm")
                for m in range(4):
                    for kc in range(4):
                        w = WT["gw2"][:, ((i * 4 + kc) * 4 + m) * 128:
                                      ((i * 4 + kc) * 4 + m + 1) * 128]
                        nc.tensor.matmul(ps2[:, m, :], w, f_bf[:, kc, :],
                                         start=(kc == 0), stop=(kc == 3))
                f2 = wp.tile([128, 4, 2], BF16, tag=f"f2_{i}")
                nc.vector.tensor_copy(f2[:], ps2[:])
                f2s.append(f2)
            # dec: low <- f3 (i=2), mid <- f2 (i=1), high <- f1 (i=0)
            for nm, cpc, fi in (("l", CPC_L, 2), ("m", CPC_M, 1), ("h", CPC_H, 0)):
                mcs = [(m0, min(128, cpc - m0)) for m0 in range(0, cpc, 128)]
                for m0, mw in mcs:
                    psd = pp.tile([128, 2], F32, tag="mm")
                    for kc in range(4):
                        w = WT["dw" + nm][:, kc * cpc + m0: kc * cpc + m0 + mw]
                        nc.tensor.matmul(psd[:mw, :], w, f2s[fi][:, kc, :],
                                         start=(kc == 0), stop=(kc == 3))
                    ot = sm.tile([128, 2], F32, tag="ot")
                    nc.scalar.copy(ot[:mw, :], psd[:mw, :])
                    nc.sync.dma_start(out=outs["o" + nm][m0:m0 + mw, :],
                                      in_=ot[:mw, :])
    nc.compile()
    return nc


# ================================================================ host packing
def _prep_unit(sampled_b, enhT_b, order_b, win):
    """Build xa/xb/mask/nmask/invn for one (b,res) unit."""
    n = sampled_b.shape[0]
    xin = np.zeros((134, LSEQ), np.float32)
    xin[0:6, 1:n + 1] = sampled_b[order_b].T
    xin[6:134, 1:n + 1] = enhT_b
    # oip_e column: v solving win.T @ v = oip_e
    return xin


def _pack_l2_core(xin, v_e, n, enc_p):
    L = LSEQ
    xin = xin.copy()
    xin[:, n + 1] = v_e
    mask = np.zeros((128, L), np.float32); mask[:, 1:n + 1] = 1.0
    smask = np.zeros((128, L), np.float32); smask[:, 0:n + 2] = 1.0

    m = {"xa": _bf(xin[0:128]), "xb": _bf(xin[128:134]),
         "mask": _bf(mask), "smask": _bf(smask),
         "invn": _f32(np.full((128, 1), 1.0 / n))}
    w_in = _f32(enc_p['in']['w'])
    m["win_a"] = _bf(w_in[0:128]); m["win_b"] = _bf(w_in[128:134])
    for k, blk in enumerate(enc_p['blocks']):
        m[f"ln_g{k}"] = _col(blk['ln_g']); m[f"ln_b{k}"] = _col(blk['ln_b'])
        m[f"wim{k}"] = _bf(blk['in']['w'])
        m[f"cw{k}"] = _f32(blk['conv_w']); m[f"cb{k}"] = _col(blk['conv_b'])
        m[f"xpw{k}"] = _bf(blk['xp_w'])
        m[f"dtw{k}"] = _bf(blk['dt']['w'])       # [R=8, di=128] is lhsT already
        m[f"dtbn{k}"] = _col(-np.asarray(blk['dt']['b'], np.float32))
        m[f"dp2{k}"] = _col(2.0 * np.asarray(blk['Dp'], np.float32))
        m[f"wout{k}"] = _bf(blk['out']['w']); m[f"wob{k}"] = _col(blk['out']['b'])
    m["norm_g"] = _col(enc_p['norm_g']); m["norm_b"] = _col(enc_p['norm_b'])
    seln = np.zeros((40, 32 * 128), np.float32)
    for idx in range(32):
        seln[8 + idx, idx * 128:(idx + 1) * 128] = 1.0
    m["sel"] = _bf(seln)
    pw1 = _f32(enc_p['out']['l1']['w'])
    m["pw1a"] = _bf(pw1[0:128]); m["pw1b"] = _bf(pw1[128:256])
    pw2 = _f32(enc_p['out']['l2']['w'])
    m["pw2"] = _bf(np.concatenate([pw2[kc * 128:(kc + 1) * 128, :]
                                   for kc in range(4)], axis=1))
    return m


def _dummy_l2_core():
    z = {n: np.zeros(s, BF if d == BF16 else np.float32)
         for n, s, d in _l2_weight_decls()}
    z["invn"][:] = 1.0
    return z


def kernel(arch_points, arch_blocks, params):
    params = _np_tree(params)
    arch_blocks = np.asarray(arch_blocks, np.float32)

    res_cfg = [("low", LOW), ("mid", MID), ("high", HIGH)]
    units = []           # list of dicts in order (res, b)
    pair_feats = []      # [7, n*32] per unit
    for rname, total in res_cfg:
        per = max(1, total // T)
        sampled = _fps(arch_blocks, per)            # [B, N, 6]
        xyz, nrm = sampled[..., :3], sampled[..., 3:6]
        knn = _knn(xyz)
        order = _zorder(xyz)
        for b in range(B):
            o = order[b]
            nb = knn[b][o]                           # [N, 32] (z-ordered rows)
            rel = xyz[b][nb] - xyz[b][o][:, None, :]
            nnb = nrm[b][nb]
            var = np.abs(1.0 - np.sum(nrm[b][o][:, None, :] * nnb, -1))
            pf = np.concatenate([rel.reshape(-1, 3),
                                 nnb.reshape(-1, 3),
                                 var.reshape(-1, 1)], axis=1).T  # [7, N*32]
            pair_feats.append(pf.astype(np.float32))
            units.append({"res": rname, "b": b, "n": sampled.shape[1],
                          "sampled": sampled[b], "order": o})

    # ---- launch 1
    allpairs = np.concatenate(pair_feats, axis=1)    # [7, NPTS*32]
    run1 = _runner("l1", _build_l1)
    enh_p = params['enh']
    w1 = {"w1c": _bf(enh_p['coord']['l1']['w']),
          "w1n": _bf(enh_p['normal']['l1']['w']),
          "w2c": _bf(enh_p['coord']['l2']['w']),
          "w2n": _bf(enh_p['normal']['l2']['w']),
          "w3a": _bf(enh_p['out']['l1']['w'][0:128]),
          "w3b": _bf(enh_p['out']['l1']['w'][128:256]),
          "w4": _bf(enh_p['out']['l2']['w'])}
    in_maps1 = []
    for c in range(NCORES):
        sl = slice(c * PAIRS_PER_CORE, (c + 1) * PAIRS_PER_CORE)
        pr = np.zeros((36, PAIRS_PER_CORE), np.float32)
        pr[0:3] = allpairs[0:3, sl]
        pr[32:36] = allpairs[3:7, sl]
        in_maps1.append({"pr": _bf(pr), **w1})
    res1 = run1(in_maps1)
    enhT = np.concatenate([res1[c]["enh"] for c in range(NCORES)], axis=1)  # [128, NPTS]

    # ---- launch 2
    run2 = _runner("l2", _build_l2)
    in_maps2 = []
    off = 0
    enc_key = {"low": "enc_low", "mid": "enc_mid", "high": "enc_high"}
    for u in units:
        n = u["n"]
        enc_p = params[enc_key[u["res"]]]
        xin = _prep_unit(u["sampled"], enhT[:, off:off + n], u["order"],
                         enc_p['in']['w'])
        off += n
        oip_e = (np.asarray(enc_p['oip']['w'], np.float32)[0]
                 + np.asarray(enc_p['oip']['b'], np.float32))
        w_in = _f32(enc_p['in']['w'])
        v_e = np.linalg.lstsq(w_in.T, oip_e - _f32(enc_p['in']['b']),
                              rcond=None)[0]
        in_maps2.append(_pack_l2_core(xin, v_e, n, enc_p))
    while len(in_maps2) < NCORES:
        in_maps2.append(_dummy_l2_core())
    res2 = run2(in_maps2)
    eo = [res2[c]["eo"].T.flatten() for c in range(6)]   # [512] each
    # units order: low_b0, low_b1, mid_b0, mid_b1, high_b0, high_b1
    gf = np.stack([np.concatenate([eo[0], eo[2], eo[4]]),
                   np.concatenate([eo[1], eo[3], eo[5]])], axis=0)  # [2,1536]

    # ---- launch 3
    run3 = _runner("l3", _build_l3)
    gfT = np.zeros((128, 24), np.float32)
    for kc in range(12):
        gfT[:, kc * 2:(kc + 1) * 2] = gf[:, kc * 128:(kc + 1) * 128].T
    gw1 = np.zeros((128, 3 * 12 * 512), np.float32)
    for i, gk in enumerate(("g1", "g2", "g3")):
        W1 = _f32(params[gk]['l1']['w'])
        for kc in range(12):
            for mch in range(4):
                gw1[:, ((i * 12 + kc) * 4 + mch) * 128:((i * 12 + kc) * 4 + mch + 1) * 128] = \
                    W1[kc * 128:(kc + 1) * 128, mch * 128:(mch + 1) * 128]
    gw2 = np.zeros((128, 3 * 4 * 512), np.float32)
    for i, gk in enumerate(("g1", "g2", "g3")):
        W2 = _f32(params[gk]['l2']['w'])
        for kc in range(4):
            for mch in range(4):
                gw2[:, ((i * 4 + kc) * 4 + mch) * 128:((i * 4 + kc) * 4 + mch + 1) * 128] = \
                    W2[kc * 128:(kc + 1) * 128, mch * 128:(mch + 1) * 128]
    in_maps3 = []
    dec = params['dec']
    for c in range(NCORES):
        m = {"gfT": _bf(gfT), "gw1": _bf(gw1), "gw2": _bf(gw2)}
        for nm, key, cpc in (("dwl", "low", CPC_L), ("dwm", "mid", CPC_M),
                             ("dwh", "high", CPC_H)):
            W = _f32(dec[key]['w'])                  # [512, LOWx3]
            sl = W[:, c * cpc:(c + 1) * cpc]
            m[nm] = _bf(np.concatenate([sl[kc * 128:(kc + 1) * 128, :]
                                        for kc in range(4)], axis=1))
        in_maps3.append(m)
    res3 = run3(in_maps3)
    dlo = np.concatenate([res3[c]["ol"].T for c in range(NCORES)], axis=1)
    dmd = np.concatenate([res3[c]["om"].T for c in range(NCORES)], axis=1)
    dhi = np.concatenate([res3[c]["oh"].T for c in range(NCORES)], axis=1)

    pl = dlo.reshape(B, LOW, 3)
    pm = _expand(pl, MID) + dmd.reshape(B, MID, 3)
    ph = _expand(pm, HIGH) + dhi.reshape(B, HIGH, 3)
    return (pl.astype(np.float32), pm.astype(np.float32),
            ph.astype(np.float32), gf.astype(np.float32))
